# revision 1
# baseline (speedup 1.0000x reference)
"""Bass/Trainium2 kernel for nn_ButterflyGatingUnit.

Data-parallel over batch B=8 across 8 NeuronCores (one image per core).

Per core:
  - The five input convs (y1, y2, q, k, v) run as im2col matmuls: the
    contraction over (tap, cin) is packed into 128-row chunks (7 per
    input half) so the PE contracts over all 128 partitions instead of
    96. im2col tiles are built per 12-row output region by contiguous
    DRAM->SBUF DMAs from three host-prepared column-shifted copies of
    each input.
  - Block-local channel attention: q/k conv outputs are written tap-major
    and DMA-transposed once; per-tap 96x96 grams accumulate over 8
    128-row chunks; softmax runs on ACT/DVE in fp32; attn @ v is a
    dynamic-weight 3x3 tap conv over the padded v.
  - BatchNorm statistics are per-core partial sums, combined with one
    tiny AllReduce per stats group (y1/y2, then y3).
  - Epilogue: out is pre-filled with x by DMA; bn terms are accumulated
    on top with SWDGE accumulate-DMAs.

All heavy matmuls are bf16 with fp32 PSUM accumulation; stats, softmax,
BN, and the gating epilogue are fp32.
"""

import os
import sys

sys.path.insert(0, "/opt/trn_rl_repo")

import numpy as np

import concourse.bass as bass
import concourse.mybir as mybir
import concourse.tile as tile
from concourse.masks import make_identity

F32 = mybir.dt.float32
BF16 = mybir.dt.bfloat16
AF = mybir.ActivationFunctionType
ALU = mybir.AluOpType
AX = mybir.AxisListType

B, C, H, W = 8, 96, 96, 96
HW = H * W                 # 9216
HP = H + 2                 # 98
NPIX = float(B * HW)       # BN count over (B,H,W)
EPS = 1e-5
S_ATTN = float(np.sqrt(C * 9.0))   # softmax scale sqrt(864)
NROW4 = 4                  # rows per y/v/attn psum tile (N = 384)
NG4 = H // NROW4           # 24 groups
NROW3 = 3                  # rows per q/k psum tile (N = 288)
FCH = 1536                 # epilogue chunk (free dim)
NFC = HW // FCH            # 6 chunks
NFILL = 8                  # 12-row im2col fills
RFREE = 12 * W             # im2col free elems per chunk per fill

# ---------------------------------------------------------------------------
# Workaround for this walrus build: only ONE sem wait is encodable per
# instruction. After Tile assigns waits, move extras onto fresh same-engine
# NoOps inserted right before the instruction (same engine + program order
# => identical blocking semantics).
_MAXW = 1


def _split_multiwaits(nc):
    for f in nc.m.functions:
        for bb in f.blocks:
            insts = bb.instructions
            if not any(
                i.sync_info is not None and len(i.sync_info.on_wait) > _MAXW
                for i in insts
            ):
                continue
            new = []
            for inst in insts:
                si = inst.sync_info
                if si is not None and len(si.on_wait) > _MAXW:
                    waits = list(si.on_wait)
                    keep, rest = waits[:_MAXW], waits[_MAXW:]
                    while rest:
                        nop = mybir.InstNoOp(name=f"I-waitsplit-{nc.next_id()}")
                        nop.engine = inst.engine
                        nop.sync_info = mybir.SyncInfo(
                            on_wait=rest[:_MAXW], on_update=[]
                        )
                        rest = rest[_MAXW:]
                        new.append(nop)
                    inst.sync_info = mybir.SyncInfo(
                        on_wait=keep, on_update=list(si.on_update)
                    )
                new.append(inst)
            bb.instructions = new


_orig_drain_and_barrier = tile.TileContext._drain_and_barrier


def _patched_drain_and_barrier(self, tick_clock, wait_clock):
    _orig_drain_and_barrier(self, tick_clock, wait_clock)
    _split_multiwaits(self.nc)


tile.TileContext._drain_and_barrier = _patched_drain_and_barrier
# ---------------------------------------------------------------------------


def _im_fill(nc, IM, xsh, base_row, row_lo, row_hi, ring):
    """Fill im2col tile IM[p, k, (r', w)] for output rows
    [base_row+row_lo, base_row+row_hi): row u = k*128+p = t*96+ci holds
    xsh[tx][ci, base_row+ty+r', w] (the host pre-shifted the kx taps into
    three column-shifted copies, so every piece is one contiguous run per
    partition). DMAs round-robin over the sync/scalar/gpsimd rings."""
    engines = [nc.sync, nc.scalar, nc.gpsimd]
    for t in range(9):
        ty, tx = t // 3, t % 3
        u0 = t * 96
        k0, p0 = u0 // 128, u0 % 128
        n0 = min(96, 128 - p0)
        pieces = [(k0, p0, 0, n0)]
        if n0 < 96:
            pieces.append((k0 + 1, 0, n0, 96 - n0))
        for (k, p, ci0, n) in pieces:
            eng = engines[ring[0] % 3]
            ring[0] += 1
            eng.dma_start(
                IM[p : p + n, k, row_lo * W : row_hi * W],
                xsh[tx][ci0 : ci0 + n,
                        base_row + ty + row_lo : base_row + ty + row_hi, :],
            )


def build_nc_v2(collectives=True):
    nc = bass.Bass(num_devices=8)

    x1s_d = [nc.dram_tensor(f"x1s{j}", [C, H + 2, W], BF16, kind="ExternalInput")
             for j in range(3)]
    x2s_d = [nc.dram_tensor(f"x2s{j}", [C, H + 2, W], BF16, kind="ExternalInput")
             for j in range(3)]
    x1f_d = nc.dram_tensor("x1f", [C, HW], F32, kind="ExternalInput")
    x2f_d = nc.dram_tensor("x2f", [C, HW], F32, kind="ExternalInput")
    w1i_d = nc.dram_tensor("w1i", [128, 7, C], BF16, kind="ExternalInput")
    w2i_d = nc.dram_tensor("w2i", [128, 7, C], BF16, kind="ExternalInput")
    wqi_d = nc.dram_tensor("wqi", [128, 2, 7, C], BF16, kind="ExternalInput")
    wki_d = nc.dram_tensor("wki", [128, 2, 7, C], BF16, kind="ExternalInput")
    wvi_d = nc.dram_tensor("wvi", [128, 2, 7, C], BF16, kind="ExternalInput")
    out_d = nc.dram_tensor("out", [2 * C, HW], F32, kind="ExternalOutput")

    with tile.TileContext(nc) as tc:
        with (
            tc.tile_pool(name="cst", bufs=1) as cst,
            tc.tile_pool(name="im", bufs=3) as im,
            tc.tile_pool(name="reuse", bufs=4) as reuse,
            tc.tile_pool(name="ystage", bufs=2) as ystage,
            tc.tile_pool(name="scr", bufs=1) as scr,
            tc.tile_pool(name="ps", bufs=4, space="PSUM") as ps,
            tc.tile_pool(name="ps3", bufs=3, space="PSUM") as ps3,
            tc.tile_pool(name="dram", bufs=1, space="DRAM") as dram,
        ):
            w1i = cst.tile([128, 7, C], BF16)
            nc.scalar.dma_start(w1i[:], w1i_d[:])
            w2i = cst.tile([128, 7, C], BF16)
            nc.scalar.dma_start(w2i[:], w2i_d[:])
            wqi = cst.tile([128, 2, 7, C], BF16)
            nc.scalar.dma_start(wqi[:], wqi_d[:])
            wki = cst.tile([128, 2, 7, C], BF16)
            nc.scalar.dma_start(wki[:], wki_d[:])
            wvi = cst.tile([128, 2, 7, C], BF16)
            nc.scalar.dma_start(wvi[:], wvi_d[:])

            y1d = dram.tile([C, HW], F32)
            y2d = dram.tile([C, HW], F32)

            qtap = reuse.tile([C, 9, 32 * 32], BF16, tag="reuse")
            ktap = reuse.tile([C, 9, 32 * 32], BF16, tag="reuse")

            vpad = cst.tile([C, HP, HP], BF16)
            nc.gpsimd.memset(vpad[:, 0, :], 0.0)
            nc.gpsimd.memset(vpad[:, HP - 1, :], 0.0)
            nc.gpsimd.memset(vpad[:, 1 : HP - 1, 0], 0.0)
            nc.gpsimd.memset(vpad[:, 1 : HP - 1, HP - 1], 0.0)

            y3b = cst.tile([C, HW], BF16)
            st = cst.tile([C, 6, NG4], F32)
            stq = cst.tile([C, 6, NG4], F32)

            def mm_chunks(pt, lhs_of, IMs, fsl, halves):
                nh = len(halves)
                i_last = nh * 7 - 1
                i = 0
                for hi, h in enumerate(halves):
                    for k in range(7):
                        Kk = 128 if k < 6 else 96
                        nc.tensor.matmul(
                            pt[:],
                            lhs_of(h, k)[0:Kk, :],
                            IMs[hi][0:Kk, k, fsl],
                            start=(i == 0),
                            stop=(i == i_last),
                        )
                        i += 1

            ring = [0]
            for r in range(NFILL):
                IM1 = im.tile([128, 7, RFREE], BF16, tag="im", name="im1")
                if r == 0:
                    # split the very first fill so the first psum group can
                    # start after a sliver of data has landed
                    _im_fill(nc, IM1, x1s_d, 0, 0, 4, ring)
                    _im_fill(nc, IM1, x1s_d, 0, 4, 12, ring)
                else:
                    _im_fill(nc, IM1, x1s_d, 12 * r, 0, 12, ring)
                IM2 = im.tile([128, 7, RFREE], BF16, tag="im", name="im2")
                _im_fill(nc, IM2, x2s_d, 12 * r, 0, 12, ring)

                # ---- y1 (x1 only) ----
                stage1 = ystage.tile([C, 12 * W], F32, tag="ystage",
                                     name="ystage")
                for i in range(3):
                    ptf = ps.tile([C, NROW4 * W], F32, tag="convps",
                                  name="convps")
                    pt = ptf[:, :]
                    mm_chunks(pt, lambda h, k: w1i[:, k, :], [IM1],
                              slice(i * 384, (i + 1) * 384), [0])
                    g = 3 * r + i
                    nc.scalar.activation(
                        out=stage1[:, i * 384 : (i + 1) * 384], in_=pt[:],
                        func=AF.Copy, accum_out=st[:, 0, g : g + 1],
                    )
                    sq = scr.tile([C, NROW4 * W], F32, tag="sqscr", name="sq")
                    nc.scalar.activation(
                        out=sq[:], in_=pt[:], func=AF.Square,
                        accum_out=stq[:, 0, g : g + 1],
                    )
                nc.gpsimd.dma_start(
                    y1d[:, r * 12 * W : (r + 1) * 12 * W], stage1[:]
                )

                # ---- q / k (both halves) ----
                for conv_i, (wt, tap) in enumerate(((wqi, qtap), (wki, ktap))):
                    for i in range(4):
                        ptf = ps3.tile([C, NROW3 * W], F32, tag="qk", name="qk")
                        pt = ptf[:, :]
                        mm_chunks(pt, lambda h, k, wt=wt: wt[:, h, k, :],
                                  [IM1, IM2], slice(i * 288, (i + 1) * 288),
                                  [0, 1])
                        g3 = 4 * r + i
                        src = pt[:].rearrange("p (ty bj tx) -> p ty tx bj",
                                              ty=3, tx=3)
                        dst = tap[:, :, g3 * 32 : (g3 + 1) * 32].rearrange(
                            "p (ty tx) l -> p ty tx l", ty=3
                        )
                        if conv_i == 0:
                            nc.scalar.activation(out=dst, in_=src, func=AF.Copy)
                        else:
                            nc.vector.tensor_copy(dst, src)

                # ---- v (both halves) ----
                for i in range(3):
                    ptf = ps.tile([C, NROW4 * W], F32, tag="convps",
                                  name="convps")
                    pt = ptf[:, :]
                    mm_chunks(pt, lambda h, k: wvi[:, h, k, :], [IM1, IM2],
                              slice(i * 384, (i + 1) * 384), [0, 1])
                    row0 = 1 + 12 * r + 4 * i
                    dst = vpad[:, row0 : row0 + 4, 1 : 1 + W]
                    if i % 2 == 0:
                        nc.scalar.activation(out=dst, in_=pt[:], func=AF.Copy)
                    else:
                        nc.vector.tensor_copy(dst, pt[:])

                # ---- y2 (x2 only; last so the next fill's IM2 DMAs get the
                # largest window before their slot is needed) ----
                stage2 = ystage.tile([C, 12 * W], F32, tag="ystage",
                                     name="ystage")
                for i in range(3):
                    ptf = ps.tile([C, NROW4 * W], F32, tag="convps",
                                  name="convps")
                    pt = ptf[:, :]
                    mm_chunks(pt, lambda h, k: w2i[:, k, :], [IM2],
                              slice(i * 384, (i + 1) * 384), [0])
                    g = 3 * r + i
                    nc.scalar.activation(
                        out=stage2[:, i * 384 : (i + 1) * 384], in_=pt[:],
                        func=AF.Copy, accum_out=st[:, 1, g : g + 1],
                    )
                    sq = scr.tile([C, NROW4 * W], F32, tag="sqscr", name="sq")
                    nc.scalar.activation(
                        out=sq[:], in_=pt[:], func=AF.Square,
                        accum_out=stq[:, 1, g : g + 1],
                    )
                nc.gpsimd.dma_start(
                    y2d[:, r * 12 * W : (r + 1) * 12 * W], stage2[:]
                )

            # ---------------- stats collective #1 (y1, y2) --------------
            stats1 = cst.tile([C, 4], F32)
            nc.vector.reduce_sum(stats1[:, 0:1], st[:, 0, :], axis=AX.X)
            nc.vector.reduce_sum(stats1[:, 1:2], stq[:, 0, :], axis=AX.X)
            nc.vector.reduce_sum(stats1[:, 2:3], st[:, 1, :], axis=AX.X)
            nc.vector.reduce_sum(stats1[:, 3:4], stq[:, 1, :], axis=AX.X)
            cc1_in = dram.tile([C, 4], F32)
            cc1_out = dram.tile([C, 4], F32)
            nc.gpsimd.dma_start(cc1_in[:], stats1[:])
            if collectives:
                nc.gpsimd.collective_compute(
                    "AllReduce", ALU.add, replica_groups=[list(range(8))],
                    ins=[cc1_in[:].opt()], outs=[cc1_out[:].opt()],
                )
            else:
                nc.gpsimd.dma_start(cc1_out[:], cc1_in[:])
            stats1r = cst.tile([C, 4], F32)
            nc.gpsimd.dma_start(stats1r[:], cc1_out[:])

            def bn_coeffs(sum_col, sq_col, label):
                mu = cst.tile([C, 1], F32, tag=f"mu_{label}", name=f"mu_{label}")
                nc.vector.tensor_scalar_mul(mu[:], sum_col, 1.0 / NPIX)
                ex2 = cst.tile([C, 1], F32, tag=f"e2_{label}", name=f"e2_{label}")
                nc.vector.tensor_scalar_mul(ex2[:], sq_col, 1.0 / NPIX)
                var = cst.tile([C, 1], F32, tag=f"v_{label}", name=f"v_{label}")
                nc.vector.tensor_tensor(var[:], mu[:], mu[:], ALU.mult)
                nc.vector.tensor_tensor(var[:], ex2[:], var[:], ALU.subtract)
                nc.vector.tensor_scalar_add(var[:], var[:], EPS)
                sd = cst.tile([C, 1], F32, tag=f"s_{label}", name=f"s_{label}")
                nc.scalar.activation(sd[:], var[:], AF.Sqrt)
                r_ = cst.tile([C, 1], F32, tag=f"r_{label}", name=f"r_{label}")
                nc.vector.reciprocal(r_[:], sd[:])
                mb = cst.tile([C, 1], F32, tag=f"m_{label}", name=f"m_{label}")
                nc.vector.tensor_scalar(
                    out=mb[:], in0=mu[:], scalar1=r_[:], scalar2=-1.0,
                    op0=ALU.mult, op1=ALU.mult,
                )
                return mu, r_, mb

            mu1, r1, mb1 = bn_coeffs(stats1r[:, 0:1], stats1r[:, 1:2], "y1")
            mu2, r2, mb2 = bn_coeffs(stats1r[:, 2:3], stats1r[:, 3:4], "y2")

            # pre-fill out with x in chunks (epilogue accum-DMAs add the bn
            # terms); scalar ring, plenty of slack before the accums land
            for c0 in range(NFC):
                sl = slice(c0 * FCH, (c0 + 1) * FCH)
                nc.scalar.dma_start(out_d[0:C, sl], x1f_d[:, sl])
                nc.scalar.dma_start(out_d[C : 2 * C, sl], x2f_d[:, sl])

            qT = reuse.tile([128, 72, C], BF16, tag="reuse")
            nc.sync.dma_start_transpose(qT[:], qtap[:].rearrange("p a b -> p (a b)"))
            kT = reuse.tile([128, 72, C], BF16, tag="reuse")
            nc.sync.dma_start_transpose(kT[:], ktap[:].rearrange("p a b -> p (a b)"))

            # ------- grams -> exp -> transpose, pipelined per tap -------
            # The softmax denominator is folded into the attn@v evacuation
            # (out3 = rd[c] * sum_t E_t^T @ v_t with rd a per-partition
            # scalar), so each tap's exp+transpose follows its gram and the
            # serial softmax block leaves the critical path. No max shift:
            # logits/sqrt(864) are O(few), safe for fp32 exp.
            A = cst.tile([C, 9, C], F32)
            attnT = cst.tile([C, 9, C], BF16)
            identf = cst.tile([C, C], F32)
            make_identity(nc, identf)
            for t in range(9):
                pgf = ps3.tile([C, NROW3 * W], F32, tag="qk", name="qk")
                pg = pgf[:, 0:C]
                for ch in range(8):
                    nc.tensor.matmul(
                        pg, kT[:, t * 8 + ch, :], qT[:, t * 8 + ch, :],
                        start=(ch == 0), stop=(ch == 7),
                    )
                # evacuate directly as E_t = exp(gram/sqrt(864))
                nc.scalar.activation(A[:, t, :], pg, AF.Exp, scale=1.0 / S_ATTN)
                ppf = ps.tile([C, NROW4 * W], F32, tag="convps", name="convps")
                pp = ppf[:, 0:C]
                nc.tensor.transpose(pp, A[:, t, :], identf[:])
                nc.scalar.activation(out=attnT[:, t, :], in_=pp, func=AF.Copy)

            Aflat = A[:].rearrange("p a b -> p (a b)")
            dsum = cst.tile([C, 1], F32)
            nc.vector.reduce_sum(dsum[:], Aflat, axis=AX.X)
            rd = cst.tile([C, 1], F32)
            nc.vector.reciprocal(rd[:], dsum[:])

            # ---------------- attn @ v (dynamic-weight tap conv) --------
            for g in range(NG4):
                ptf = ps.tile([C, NROW4 * W], F32, tag="convps", name="convps")
                pt = ptf[:, :]
                for t in range(9):
                    ky, kx = t // 3, t % 3
                    rhs = vpad[:, NROW4 * g + ky : NROW4 * g + ky + NROW4,
                               kx : kx + W]
                    nc.tensor.matmul(
                        pt[:], attnT[:, t, :], rhs, start=(t == 0), stop=(t == 8)
                    )
                sl = slice(g * NROW4 * W, (g + 1) * NROW4 * W)
                nc.vector.tensor_scalar(
                    out=y3b[:, sl], in0=pt[:], scalar1=rd[:], scalar2=0.0,
                    op0=ALU.mult, op1=ALU.add,
                    accum_out=st[:, 2, g : g + 1],
                )
                sq = scr.tile([C, NROW4 * W], F32, tag="sqscr", name="sq")
                nc.scalar.activation(
                    out=sq[:], in_=pt[:], func=AF.Square, scale=rd[:],
                    accum_out=stq[:, 2, g : g + 1],
                )

            # ---------------- stats collective #2 (y3) ----------------
            stats2 = cst.tile([C, 2], F32)
            nc.vector.reduce_sum(stats2[:, 0:1], st[:, 2, :], axis=AX.X)
            nc.vector.reduce_sum(stats2[:, 1:2], stq[:, 2, :], axis=AX.X)
            cc2_in = dram.tile([C, 2], F32)
            cc2_out = dram.tile([C, 2], F32)
            nc.gpsimd.dma_start(cc2_in[:], stats2[:])
            if collectives:
                nc.gpsimd.collective_compute(
                    "AllReduce", ALU.add, replica_groups=[list(range(8))],
                    ins=[cc2_in[:].opt()], outs=[cc2_out[:].opt()],
                )
            else:
                nc.gpsimd.dma_start(cc2_out[:], cc2_in[:])
            stats2r = cst.tile([C, 2], F32)
            nc.gpsimd.dma_start(stats2r[:], cc2_out[:])
            mu3, r3, mb3 = bn_coeffs(stats2r[:, 0:1], stats2r[:, 1:2], "y3")

            # ---------------- epilogue ----------------
            # out[0:96]   = x1 + bn(y1)*bn(y2)   (x pre-filled, bn accum'd)
            # out[96:192] = x2 + bn(y3)
            for c0 in range(NFC):
                sl = slice(c0 * FCH, (c0 + 1) * FCH)
                ry1 = im.tile([C, FCH], F32, tag="im", name="ry1")
                nc.sync.dma_start(ry1[:], y1d[:, sl])
                ry2 = im.tile([C, FCH], F32, tag="im", name="ry2")
                nc.sync.dma_start(ry2[:], y2d[:, sl])
                t1 = reuse.tile([C, FCH], F32, tag="reuse")
                nc.scalar.activation(
                    t1[:], ry1[:], AF.Identity, bias=mb1[:], scale=r1[:]
                )
                t2 = reuse.tile([C, FCH], F32, tag="reuse")
                nc.vector.tensor_scalar(
                    out=t2[:], in0=ry2[:], scalar1=mu2[:], scalar2=r2[:],
                    op0=ALU.subtract, op1=ALU.mult,
                )
                g12 = reuse.tile([C, FCH], F32, tag="reuse")
                nc.vector.tensor_tensor(g12[:], t1[:], t2[:], ALU.mult)
                nc.gpsimd.dma_start(out_d[0:C, sl], g12[:], accum_op=ALU.add)

            for c0 in range(NFC):
                sl = slice(c0 * FCH, (c0 + 1) * FCH)
                t3 = reuse.tile([C, FCH], F32, tag="reuse")
                if c0 % 2 == 0:
                    nc.scalar.activation(
                        t3[:], y3b[:, sl], AF.Identity, bias=mb3[:], scale=r3[:]
                    )
                else:
                    nc.vector.tensor_scalar(
                        out=t3[:], in0=y3b[:, sl], scalar1=mu3[:], scalar2=r3[:],
                        op0=ALU.subtract, op1=ALU.mult,
                    )
                nc.gpsimd.dma_start(out_d[C : 2 * C, sl], t3[:], accum_op=ALU.add)

    return nc


# alias for profiling scripts
build_nc = build_nc_v2

_CACHED_NC = None


def _get_nc():
    global _CACHED_NC
    if _CACHED_NC is None:
        _CACHED_NC = build_nc_v2()
    return _CACHED_NC


def _host_prep(x1, x2, w1, w2, wa1, wa2, wa3):
    import ml_dtypes

    bf = ml_dtypes.bfloat16

    x1f = np.ascontiguousarray(x1.reshape(B, C, HW)).astype(np.float32)
    x2f = np.ascontiguousarray(x2.reshape(B, C, HW)).astype(np.float32)

    def shifted(x):
        # xs[tx][b, ci, row, w] = row/col zero-padded x at column w+tx
        xp = np.zeros((B, C, H + 2, W + 2), np.float32)
        xp[:, :, 1 : 1 + H, 1 : 1 + W] = x
        return [np.ascontiguousarray(xp[:, :, :, j : j + W]).astype(bf)
                for j in range(3)]

    x1s = shifted(x1)
    x2s = shifted(x2)

    def w_im_half(w):
        # [Cout, 96, 3, 3] -> [128, 7, Cout] with row u = t*96+ci (padded)
        co, ci = w.shape[0], w.shape[1]
        u = np.transpose(w.reshape(co, ci, 9), (2, 1, 0)).reshape(9 * ci, co)
        up = np.zeros((896, co), np.float32)
        up[: 9 * ci] = u
        return np.ascontiguousarray(
            up.reshape(7, 128, co).transpose(1, 0, 2)
        ).astype(bf)

    weights = {
        "w1i": w_im_half(w1),
        "w2i": w_im_half(w2),
        "wqi": np.stack([w_im_half(wa1[:, :C]), w_im_half(wa1[:, C:])], 1),
        "wki": np.stack([w_im_half(wa2[:, :C]), w_im_half(wa2[:, C:])], 1),
        "wvi": np.stack([w_im_half(wa3[:, :C]), w_im_half(wa3[:, C:])], 1),
    }
    in_maps = []
    for b in range(B):
        m = {"x1f": x1f[b], "x2f": x2f[b]}
        for j in range(3):
            m[f"x1s{j}"] = x1s[j][b]
            m[f"x2s{j}"] = x2s[j][b]
        m.update(weights)
        in_maps.append(m)
    return in_maps


def kernel(x1, x2, w1, w2, wa1, wa2, wa3):
    from concourse.bass_utils import run_bass_kernel_spmd

    x1 = np.asarray(x1, np.float32)
    x2 = np.asarray(x2, np.float32)
    in_maps = _host_prep(
        x1, x2,
        np.asarray(w1, np.float32), np.asarray(w2, np.float32),
        np.asarray(wa1, np.float32), np.asarray(wa2, np.float32),
        np.asarray(wa3, np.float32),
    )
    nc = _get_nc()
    res = run_bass_kernel_spmd(nc, in_maps, core_ids=list(range(8)))
    out0 = np.stack([res.results[b]["out"] for b in range(B)], 0).reshape(
        B, 2 * C, H, W
    )
    out1 = np.concatenate([x1, x2], axis=1)
    return out0, out1



# revision 37
# speedup vs baseline: 1.1601x; 1.1601x over previous
"""Bass/Trainium2 kernel for nn_ButterflyGatingUnit (v2).

Data-parallel over batch B=8 across 8 NeuronCores (one image per core).

v2 redesign vs v1 (307us -> target ~230us):
  - im2col is built on the HOST into two DRAM tensors (one per input half),
    with the per-fill pixel ordering PHASE-GROUPED: free index within a
    12-row fill is (ty, tx, j, bx) where the output pixel is
    (12r + 3j + ty, 3bx + tx).  One big DMA per (input, fill) replaces
    ~26 small DMAs, and every 128-slice of the free dim is exactly one
    3x3 phase class = one attention tap's block positions for that fill.
  - q and k convs run in TRANSPOSED orientation: lhsT = im2col tile
    (m = 128 pixels, full PE rows), rhs = [wq|wk] weights (free = 192).
    The psum windows come out as qT/kT chunks [128 positions, 192] that
    feed the gram matmuls directly -- no DMA transposes at all.
  - Per-tap gram matrices accumulate in two persistent PSUM banks across
    all 8 fills (one start=True per bank; the bank's pending-zero covers
    the other taps' first write), so the attention setup leaves the
    critical path.
  - y1/y2 stay in SBUF as bf16 (phase-ordered; no DRAM round trip).  The
    epilogue is all-bf16 (x is staged as bf16, out is written bf16 and
    widened on the host), computed in SBUF with the phase->raster
    un-permute folded into the final add's access patterns, engine-
    balanced so it hides under attn@v.
  - v is un-permuted to a raster padded buffer at evacuation (3 strided
    copies per psum tile on Pool); attn @ v is the same dynamic-weight
    3x3 tap conv as v1.

All heavy matmuls are bf16 with fp32 PSUM accumulation; stats, softmax,
and BN coefficients are fp32.
"""

import os
import sys

sys.path.insert(0, "/opt/trn_rl_repo")

import numpy as np

import concourse.bass as bass
import concourse.mybir as mybir
import concourse.tile as tile
from concourse.masks import make_identity

F32 = mybir.dt.float32
BF16 = mybir.dt.bfloat16
AF = mybir.ActivationFunctionType
ALU = mybir.AluOpType
AX = mybir.AxisListType

B, C, H, W = 8, 96, 96, 96
HW = H * W                 # 9216
HP = H + 2                 # 98
NPIX = float(B * HW)       # BN count over (B,H,W)
EPS = 1e-5
S_ATTN = float(np.sqrt(C * 9.0))   # softmax scale sqrt(864)
NF = 8                     # fills (12 rows each)
FILL = HW // NF            # 1152 pixels per fill
NROW4 = 4                  # rows per attn psum tile (N = 384)
NG4 = H // NROW4           # 24 attn@v groups

# ---------------------------------------------------------------------------
# Workaround for this walrus build: only ONE sem wait is encodable per
# instruction. After Tile assigns waits, move extras onto fresh same-engine
# NoOps inserted right before the instruction (same engine + program order
# => identical blocking semantics).
_MAXW = 1


def _split_multiwaits(nc):
    for f in nc.m.functions:
        for bb in f.blocks:
            insts = bb.instructions
            if not any(
                i.sync_info is not None and len(i.sync_info.on_wait) > _MAXW
                for i in insts
            ):
                continue
            new = []
            for inst in insts:
                si = inst.sync_info
                if si is not None and len(si.on_wait) > _MAXW:
                    waits = list(si.on_wait)
                    keep, rest = waits[:_MAXW], waits[_MAXW:]
                    while rest:
                        nop = mybir.InstNoOp(name=f"I-waitsplit-{nc.next_id()}")
                        nop.engine = inst.engine
                        nop.sync_info = mybir.SyncInfo(
                            on_wait=rest[:_MAXW], on_update=[]
                        )
                        rest = rest[_MAXW:]
                        new.append(nop)
                    inst.sync_info = mybir.SyncInfo(
                        on_wait=keep, on_update=list(si.on_update)
                    )
                new.append(inst)
            bb.instructions = new


_orig_drain_and_barrier = tile.TileContext._drain_and_barrier


def _patched_drain_and_barrier(self, tick_clock, wait_clock):
    _orig_drain_and_barrier(self, tick_clock, wait_clock)
    _split_multiwaits(self.nc)


tile.TileContext._drain_and_barrier = _patched_drain_and_barrier
# ---------------------------------------------------------------------------


def build_nc_v2(collectives=True):
    nc = bass.Bass(num_devices=8)

    im1_d = nc.dram_tensor("im1", [128, 7, HW], BF16, kind="ExternalInput")
    im2_d = nc.dram_tensor("im2", [128, 7, HW], BF16, kind="ExternalInput")
    x1f_d = nc.dram_tensor("x1f", [C, HW], BF16, kind="ExternalInput")
    x2f_d = nc.dram_tensor("x2f", [C, HW], BF16, kind="ExternalInput")
    w1i_d = nc.dram_tensor("w1i", [128, 7, C], BF16, kind="ExternalInput")
    w2i_d = nc.dram_tensor("w2i", [128, 7, C], BF16, kind="ExternalInput")
    wqk_d = nc.dram_tensor("wqk", [128, 2, 7, 2 * C], BF16, kind="ExternalInput")
    wvi_d = nc.dram_tensor("wvi", [128, 2, 7, C], BF16, kind="ExternalInput")
    out_d = nc.dram_tensor("out", [2 * C, HW], BF16, kind="ExternalOutput")

    with tile.TileContext(nc) as tc:
        with (
            tc.tile_pool(name="cst", bufs=1) as cst,
            tc.tile_pool(name="im", bufs=4) as im,
            tc.tile_pool(name="reuse", bufs=4) as reuse,
            tc.tile_pool(name="scr", bufs=1) as scr,
            tc.tile_pool(name="ps", bufs=4, space="PSUM") as ps,
            tc.tile_pool(name="psq", bufs=2, space="PSUM") as psq,
            tc.tile_pool(name="gram", bufs=1, space="PSUM") as gram,
            tc.tile_pool(name="dram", bufs=1, space="DRAM") as dram,
        ):
            w1i = cst.tile([128, 7, C], BF16)
            wqk = cst.tile([128, 2, 7, 2 * C], BF16)
            wvi = cst.tile([128, 2, 7, C], BF16)
            w2i = cst.tile([128, 7, C], BF16)

            # phase-ordered bf16 conv outputs kept in SBUF
            y1b = cst.tile([C, HW], BF16)
            y2b = cst.tile([C, HW], BF16)
            # qT/kT chunks: [pos-in-chunk, tap, fill, 0:96=q | 96:192=k]
            qkT = cst.tile([128, 9, NF, 2 * C], BF16)
            # raster padded v for the dynamic tap conv
            vpad = cst.tile([C, HP, HP], BF16)
            nc.gpsimd.memset(vpad[:, 0, :], 0.0)
            nc.gpsimd.memset(vpad[:, HP - 1, :], 0.0)
            nc.gpsimd.memset(vpad[:, 1 : HP - 1, 0], 0.0)
            nc.gpsimd.memset(vpad[:, 1 : HP - 1, HP - 1], 0.0)

            y3b = cst.tile([C, HW], BF16)
            st = cst.tile([C, 3, NG4], F32)
            stq = cst.tile([C, 3, NG4], F32)

            # persistent gram accumulators: taps 0-4 and 5-8
            gram1 = gram.tile([C, 5 * C], F32)
            gram2 = gram.tile([C, 4 * C], F32)

            def mm_norm(pt, lhs_of, IMs, fsl, halves):
                i_last = len(halves) * 7 - 1
                i = 0
                for hi in halves:
                    for k in range(7):
                        Kk = 128 if k < 6 else 96
                        nc.tensor.matmul(
                            pt[:],
                            lhs_of(hi, k)[0:Kk, :],
                            IMs[hi][0:Kk, k, fsl],
                            start=(i == 0),
                            stop=(i == i_last),
                        )
                        i += 1

            def emit_gram(rp, taps=range(9)):
                # one matmul per tap accumulating chunk rp into the
                # persistent gram banks; exactly one start per BANK (the
                # pending-zero region covers the other taps' first write).
                for t in taps:
                    if t < 5:
                        out_ap = gram1[:, t * C : (t + 1) * C]
                        first = rp == 0 and t == 0
                        last = rp == NF - 1 and t == 4
                    else:
                        out_ap = gram2[:, (t - 5) * C : (t - 4) * C]
                        first = rp == 0 and t == 5
                        last = rp == NF - 1 and t == 8
                    nc.tensor.matmul(
                        out_ap,
                        qkT[:, t, rp, C : 2 * C],
                        qkT[:, t, rp, 0:C],
                        start=first,
                        stop=last,
                        skip_group_check=True,
                    )

            def qk_windows(r, i, IMs):
                for t in range(3 * i, 3 * i + 3):
                    pqf = psq.tile([128, 2 * C], F32, tag="qk", name="qk")
                    pq = pqf[:, :]
                    mi = 0
                    for hi, IMh in enumerate(IMs):
                        for k in range(7):
                            Kk = 128 if k < 6 else 96
                            nc.tensor.matmul(
                                pq,
                                IMh[0:Kk, k, t * 128 : (t + 1) * 128],
                                wqk[0:Kk, hi, k, :],
                                start=(mi == 0),
                                stop=(mi == 13),
                            )
                            mi += 1
                    if t % 2 == 0:
                        nc.vector.tensor_copy(qkT[:, t, r, :], pq)
                    else:
                        nc.scalar.activation(
                            out=qkT[:, t, r, :], in_=pq, func=AF.Copy
                        )

            def conv_third(r, i, IMs, qk_first=False):
                """One 384-pixel third of a fill."""
                fsl = slice(i * 384, (i + 1) * 384)
                g = 3 * r + i
                if qk_first:
                    qk_windows(r, i, IMs)
                    emit_gram(r, range(3 * i, 3 * i + 3))
                # ---- y1 (x1 only), phase-ordered evac + stats ----
                ptf = ps.tile([C, 384], F32, tag="convps", name="convps")
                pt = ptf[:, :]
                mm_norm(pt, lambda h, k: w1i[:, k, :], IMs, fsl, [0])
                if qk_first:
                    # keep fill 7's ACT queue clear so the softmax exps fire
                    # as soon as the grams close
                    nc.vector.tensor_scalar(
                        out=y1b[:, r * FILL + i * 384 : r * FILL + (i + 1) * 384],
                        in0=pt[:], scalar1=1.0, scalar2=0.0,
                        op0=ALU.mult, op1=ALU.add,
                        accum_out=st[:, 0, g : g + 1],
                    )
                else:
                    nc.scalar.activation(
                        out=y1b[:, r * FILL + i * 384 : r * FILL + (i + 1) * 384],
                        in_=pt[:], func=AF.Copy, accum_out=st[:, 0, g : g + 1],
                    )
                y1sl = y1b[:, r * FILL + i * 384 : r * FILL + (i + 1) * 384]
                sq = scr.tile([C, 384], F32, tag="sqscr", name="sq")
                if qk_first:
                    sqb = scr.tile([C, 384], BF16, tag="sqb", name="sqb")
                    nc.vector.scalar_tensor_tensor(
                        out=sqb[:], in0=y1sl, scalar=1.0, in1=y1sl,
                        op0=ALU.mult, op1=ALU.mult,
                        accum_out=stq[:, 0, g : g + 1],
                    )
                else:
                    nc.scalar.activation(
                        out=sq[:], in_=pt[:], func=AF.Square,
                        accum_out=stq[:, 0, g : g + 1],
                    )
                if not qk_first:
                    qk_windows(r, i, IMs)
                # ---- v (both halves), un-permute to raster vpad ----
                vwin = vpad[:, 1 + 12 * r : 13 + 12 * r, 1 : 1 + W].rearrange(
                    "p (j ty) (bx tx) -> p ty tx j bx", j=4, ty=3, bx=32, tx=3
                )
                ptf = ps.tile([C, 384], F32, tag="convps", name="convps")
                pt = ptf[:, :]
                mm_norm(pt, lambda h, k: wvi[:, h, k, :], IMs, fsl, [0, 1])
                src = pt.rearrange("p (tx j bx) -> p tx j bx", tx=3, j=4, bx=32)
                if i % 2 == 0:
                    nc.vector.tensor_copy(vwin[:, i], src)
                else:
                    nc.scalar.activation(out=vwin[:, i], in_=src, func=AF.Copy)
                # ---- y2 (x2 only) ----
                ptf = ps.tile([C, 384], F32, tag="convps", name="convps")
                pt = ptf[:, :]
                mm_norm(pt, lambda h, k: w2i[:, k, :], IMs, fsl, [1])
                nc.vector.tensor_scalar(
                    out=y2b[:, r * FILL + i * 384 : r * FILL + (i + 1) * 384],
                    in0=pt[:], scalar1=1.0, scalar2=0.0,
                    op0=ALU.mult, op1=ALU.add,
                    accum_out=st[:, 1, g : g + 1],
                )
                y2sl = y2b[:, r * FILL + i * 384 : r * FILL + (i + 1) * 384]
                sq = scr.tile([C, 384], BF16, tag="sqscr", name="sq")
                nc.vector.scalar_tensor_tensor(
                    out=sq[:], in0=y2sl, scalar=1.0, in1=y2sl,
                    op0=ALU.mult, op1=ALU.mult,
                    accum_out=stq[:, 1, g : g + 1],
                )

            for r in range(NF):
                IM1 = im.tile([128, 7, FILL], BF16, tag="im", name="im1")
                IM2 = im.tile([128, 7, FILL], BF16, tag="im", name="im2")
                sl_r = slice(r * FILL, (r + 1) * FILL)
                if r == 0:
                    # interleave piecewise so compute starts after ~2us;
                    # later-needed weights load between the pieces
                    nc.sync.dma_start(IM1[:, :, 0:384], im1_d[:, :, 0:384])
                    nc.scalar.dma_start(w1i[:], w1i_d[:])
                    nc.sync.dma_start(IM2[:, :, 0:384], im2_d[:, :, 0:384])
                    nc.scalar.dma_start(wqk[:], wqk_d[:])
                    nc.sync.dma_start(IM1[:, :, 384:768], im1_d[:, :, 384:768])
                    nc.sync.dma_start(IM2[:, :, 384:768], im2_d[:, :, 384:768])
                    nc.scalar.dma_start(wvi[:], wvi_d[:])
                    nc.scalar.dma_start(w2i[:], w2i_d[:])
                    nc.sync.dma_start(IM1[:, :, 768:FILL], im1_d[:, :, 768:FILL])
                    nc.sync.dma_start(IM2[:, :, 768:FILL], im2_d[:, :, 768:FILL])
                else:
                    nc.sync.dma_start(IM1[:], im1_d[:, :, sl_r])
                    nc.sync.dma_start(IM2[:], im2_d[:, :, sl_r])
                if r == 1:
                    # prefill out halves with x; epilogue accum-DMAs add the
                    # bn terms on top
                    nc.scalar.dma_start(out_d[0:C, :], x1f_d[:])

                # gram partials for the previous fill's chunk; the last
                # fill's own partials are interleaved with its qk windows,
                # so its predecessor must be accumulated first
                for i in range(3):
                    conv_third(r, i, [IM1, IM2], qk_first=False)
                if r > 0:
                    emit_gram(r - 1)
            emit_gram(NF - 1)

            # ------- softmax: exp of grams, denominator, transposes -------
            # No max shift: logits/sqrt(864) are O(few), safe for fp32 exp.
            A = cst.tile([C, 9, C], F32)
            attnT = cst.tile([C, 9, C], BF16)
            identf = cst.tile([C, C], F32)
            make_identity(nc, identf)
            nc.scalar.activation(
                A[:, 0:5, :].rearrange("p a b -> p (a b)"), gram1[:, :],
                AF.Exp, scale=1.0 / S_ATTN,
            )
            nc.scalar.activation(
                A[:, 5:9, :].rearrange("p a b -> p (a b)"), gram2[:, :],
                AF.Exp, scale=1.0 / S_ATTN,
            )
            Aflat = A[:].rearrange("p a b -> p (a b)")
            dsum = cst.tile([C, 1], F32)
            nc.vector.reduce_sum(dsum[:], Aflat, axis=AX.X)
            rd = cst.tile([C, 1], F32)
            nc.vector.reciprocal(rd[:], dsum[:])
            for t in range(9):
                ppf = ps.tile([C, 384], F32, tag="convps", name="convps")
                pp = ppf[:, 0:C]
                nc.tensor.transpose(pp, A[:, t, :], identf[:])
                if t % 2 == 0:
                    nc.scalar.activation(out=attnT[:, t, :], in_=pp, func=AF.Copy)
                else:
                    nc.vector.tensor_copy(attnT[:, t, :], pp)

            # ---------------- stats collective #1 (y1, y2) --------------
            stats1 = cst.tile([C, 4], F32)
            nc.vector.reduce_sum(stats1[:, 0:1], st[:, 0, :], axis=AX.X)
            nc.vector.reduce_sum(stats1[:, 1:2], stq[:, 0, :], axis=AX.X)
            nc.vector.reduce_sum(stats1[:, 2:3], st[:, 1, :], axis=AX.X)
            nc.vector.reduce_sum(stats1[:, 3:4], stq[:, 1, :], axis=AX.X)
            cc1_in = dram.tile([C, 4], F32)
            cc1_out = dram.tile([C, 4], F32)
            nc.sync.dma_start(cc1_in[:], stats1[:])
            if collectives:
                nc.gpsimd.collective_compute(
                    "AllReduce", ALU.add, replica_groups=[list(range(8))],
                    ins=[cc1_in[:].opt()], outs=[cc1_out[:].opt()],
                )
            else:
                nc.sync.dma_start(cc1_out[:], cc1_in[:])
            stats1r = cst.tile([C, 4], F32)
            nc.sync.dma_start(stats1r[:], cc1_out[:])

            def bn_coeffs(sum_col, sq_col, label):
                mu = cst.tile([C, 1], F32, tag=f"mu_{label}", name=f"mu_{label}")
                nc.vector.tensor_scalar_mul(mu[:], sum_col, 1.0 / NPIX)
                ex2 = cst.tile([C, 1], F32, tag=f"e2_{label}", name=f"e2_{label}")
                nc.vector.tensor_scalar_mul(ex2[:], sq_col, 1.0 / NPIX)
                var = cst.tile([C, 1], F32, tag=f"v_{label}", name=f"v_{label}")
                nc.vector.tensor_tensor(var[:], mu[:], mu[:], ALU.mult)
                nc.vector.tensor_tensor(var[:], ex2[:], var[:], ALU.subtract)
                nc.vector.tensor_scalar_add(var[:], var[:], EPS)
                sd = cst.tile([C, 1], F32, tag=f"s_{label}", name=f"s_{label}")
                nc.scalar.activation(sd[:], var[:], AF.Sqrt)
                r_ = cst.tile([C, 1], F32, tag=f"r_{label}", name=f"r_{label}")
                nc.vector.reciprocal(r_[:], sd[:])
                mb = None
                if label in ("y2", "y3"):
                    mb = cst.tile([C, 1], F32, tag=f"m_{label}",
                                  name=f"m_{label}")
                    nc.vector.tensor_scalar(
                        out=mb[:], in0=mu[:], scalar1=r_[:], scalar2=-1.0,
                        op0=ALU.mult, op1=ALU.mult,
                    )
                return mu, r_, mb

            mu1, r1, mb1 = bn_coeffs(stats1r[:, 0:1], stats1r[:, 1:2], "y1")
            mu2, r2, mb2 = bn_coeffs(stats1r[:, 2:3], stats1r[:, 3:4], "y2")

            # ------------- attn @ v with interleaved epilogue-1 ----------
            # out[0:96] = x1 + bn(y1)*bn(y2), all bf16, computed in SBUF and
            # written once; the phase->raster un-permute rides on the final
            # add's access patterns.  Engine split per chunk: t1 ACT,
            # t2 DVE, g12 Pool, un-permute adds DVE, write on SP.
            # combined scalars: gated = (y1-mu1)*(rc*y2 + bc),
            # rc = r1*r2, bc = -mu2*rc
            rc = cst.tile([C, 1], F32)
            nc.vector.tensor_tensor(rc[:], r1[:], r2[:], ALU.mult)
            bc = cst.tile([C, 1], F32)
            nc.vector.tensor_scalar(
                out=bc[:], in0=mu2[:], scalar1=rc[:], scalar2=-1.0,
                op0=ALU.mult, op1=ALU.mult,
            )

            def epi1_chunk(c0):
                sl = slice(c0 * FILL, (c0 + 1) * FILL)
                t2 = reuse.tile([C, FILL], BF16, tag="reuse")
                nc.scalar.activation(
                    t2[:], y2b[:, sl], AF.Identity, bias=bc[:], scale=rc[:]
                )
                # g12 = (y1-mu1)*t2 with the phase->raster un-permute on the
                # dst AP (split per ty to keep APs at 4 dims)
                g12 = reuse.tile([C, FILL], BF16, tag="reuse")
                # HW limits stt outputs to <=2 free dims: iterate (ty, j),
                # each piece is [p, tx, bx]
                t1v = y1b[:, sl].rearrange("p (ty tx j bx) -> p ty j tx bx",
                                           ty=3, tx=3, j=4, bx=32)
                t2v = t2[:].rearrange("p (ty tx j bx) -> p ty j tx bx",
                                      ty=3, tx=3, j=4, bx=32)
                gv = g12[:].rearrange("p (j ty bx tx) -> p ty j tx bx",
                                      j=4, ty=3, bx=32, tx=3)
                for ty in range(3):
                    for j in range(4):
                        nc.vector.scalar_tensor_tensor(
                            out=gv[:, ty, j], in0=t1v[:, ty, j], scalar=mu1[:],
                            in1=t2v[:, ty, j], op0=ALU.subtract, op1=ALU.mult,
                        )
                nc.gpsimd.dma_start(out_d[0:C, sl], g12[:], accum_op=ALU.add)

            # prefetch x2 for the tail while PE runs attn@v (im pool is
            # free after the conv phase; exactly 4 slots)
            T3CHUNKS = [(0, 1152, "dve"), (1152, 3456, "act"),
                        (3456, 5760, "dve"), (5760, 8064, "act"),
                        (8064, 9216, "dve")]
            rx2_tiles = []
            for lo, hi, eng in T3CHUNKS[:4]:
                rx2 = im.tile([C, hi - lo], BF16, tag="im", name="rx2")
                nc.scalar.dma_start(rx2[:], x2f_d[:, lo:hi])
                rx2_tiles.append(rx2)
            rx2_tiles.append(None)

            EPI1_AT = {2: 0, 5: 1, 8: 2, 10: 3, 13: 4, 15: 5, 18: 6, 20: 7}
            for g in range(NG4):
                ptf = ps.tile([C, 384], F32, tag="convps", name="convps")
                pt = ptf[:, :]
                for t in range(9):
                    ky, kx = t // 3, t % 3
                    rhs = vpad[:, NROW4 * g + ky : NROW4 * g + ky + NROW4,
                               kx : kx + W]
                    nc.tensor.matmul(
                        pt[:], attnT[:, t, :], rhs, start=(t == 0), stop=(t == 8)
                    )
                sl = slice(g * NROW4 * W, (g + 1) * NROW4 * W)
                nc.vector.tensor_scalar(
                    out=y3b[:, sl], in0=pt[:], scalar1=rd[:], scalar2=0.0,
                    op0=ALU.mult, op1=ALU.add,
                    accum_out=st[:, 2, g : g + 1],
                )
                sq = scr.tile([C, 384], F32, tag="sqscr", name="sq")
                nc.scalar.activation(
                    out=sq[:], in_=pt[:], func=AF.Square, scale=rd[:],
                    accum_out=stq[:, 2, g : g + 1],
                )
                if g in EPI1_AT:
                    epi1_chunk(EPI1_AT[g])

            # ---------------- stats collective #2 (y3) ----------------
            stats2 = cst.tile([C, 2], F32)
            nc.vector.reduce_sum(stats2[:, 0:1], st[:, 2, :], axis=AX.X)
            nc.vector.reduce_sum(stats2[:, 1:2], stq[:, 2, :], axis=AX.X)
            cc2_in = dram.tile([C, 2], F32)
            cc2_out = dram.tile([C, 2], F32)
            nc.sync.dma_start(cc2_in[:], stats2[:])
            if collectives:
                nc.gpsimd.collective_compute(
                    "AllReduce", ALU.add, replica_groups=[list(range(8))],
                    ins=[cc2_in[:].opt()], outs=[cc2_out[:].opt()],
                )
            else:
                nc.sync.dma_start(cc2_out[:], cc2_in[:])
            stats2r = cst.tile([C, 2], F32)
            nc.sync.dma_start(stats2r[:], cc2_out[:])
            mu3, r3, mb3 = bn_coeffs(stats2r[:, 0:1], stats2r[:, 1:2], "y3")

            # ---------------- epilogue half 2: x2 + bn(y3) ----------------
            # t3 rotates ACT/DVE/Pool, adds rotate DVE/ACT? (ACT cannot add
            # two tensors) -> adds on DVE (bf16 2x) and Pool alternating.
            for (lo, hi, eng), rx2 in zip(T3CHUNKS, rx2_tiles):
                sl = slice(lo, hi)
                if rx2 is None:
                    rx2 = im.tile([C, hi - lo], BF16, tag="im", name="rx2")
                    nc.sync.dma_start(rx2[:], x2f_d[:, sl])
                t3 = reuse.tile([C, hi - lo], BF16, tag="t3", bufs=3,
                                padded_shape=[C, 2304], name="t3")
                if eng == "dve":
                    nc.vector.tensor_scalar(
                        out=t3[:], in0=y3b[:, sl], scalar1=mu3[:], scalar2=r3[:],
                        op0=ALU.subtract, op1=ALU.mult,
                    )
                else:
                    nc.scalar.activation(
                        t3[:], y3b[:, sl], AF.Identity, bias=mb3[:], scale=r3[:]
                    )
                nc.vector.tensor_tensor(t3[:], t3[:], rx2[:], ALU.add)
                nc.sync.dma_start(out_d[C : 2 * C, sl], t3[:])

    return nc


# alias for profiling scripts
build_nc = build_nc_v2

_CACHED_NC = None


def _get_nc():
    global _CACHED_NC
    if _CACHED_NC is None:
        _CACHED_NC = build_nc_v2()
    return _CACHED_NC


def _host_prep(x1, x2, w1, w2, wa1, wa2, wa3):
    import ml_dtypes

    bf = ml_dtypes.bfloat16

    x1f = np.ascontiguousarray(x1.reshape(B, C, HW)).astype(bf)
    x2f = np.ascontiguousarray(x2.reshape(B, C, HW)).astype(bf)

    def im2col_phase(x):
        # x: [B, C, H, W] fp32 -> [B, 128, 7, HW] bf16 im2col with rows
        # u = t*96 + ci (t = 3x3 conv tap, raster) packed as u = k*128 + p,
        # and free dim per 12-row fill ordered (ty, tx, j, bx) for output
        # pixel (12r + 3j + ty, 3bx + tx).
        xp = np.zeros((B, C, H + 2, W + 2), bf)
        xp[:, :, 1 : 1 + H, 1 : 1 + W] = x.astype(bf)
        out = np.zeros((B, 896, NF, 3, 3, 4, 32), bf)
        for ky in range(3):
            for kx in range(3):
                t = ky * 3 + kx
                v = xp[:, :, ky : ky + H, kx : kx + W]
                # rows: y = 12r + 3j + ty -> (r, j, ty); cols: x = 3bx+tx
                v = v.reshape(B, C, NF, 4, 3, 32, 3)
                # [B, ci, r, j, ty, bx, tx] -> [B, ci, r, ty, tx, j, bx]
                out[:, t * C : (t + 1) * C] = v.transpose(0, 1, 2, 4, 6, 3, 5)
        out = out.reshape(B, 7, 128, HW).transpose(0, 2, 1, 3)
        return np.ascontiguousarray(out)

    im1 = im2col_phase(x1)
    im2 = im2col_phase(x2)

    def w_im_half(w):
        # [Cout, 96, 3, 3] -> [128, 7, Cout] with row u = t*96+ci (padded)
        co, ci = w.shape[0], w.shape[1]
        u = np.transpose(w.reshape(co, ci, 9), (2, 1, 0)).reshape(9 * ci, co)
        up = np.zeros((896, co), np.float32)
        up[: 9 * ci] = u
        return np.ascontiguousarray(
            up.reshape(7, 128, co).transpose(1, 0, 2)
        ).astype(bf)

    def wqk_half(h):
        sl = slice(0, C) if h == 0 else slice(C, 2 * C)
        return np.concatenate([w_im_half(wa1[:, sl]), w_im_half(wa2[:, sl])],
                              axis=-1)

    weights = {
        "w1i": w_im_half(w1),
        "w2i": w_im_half(w2),
        "wqk": np.stack([wqk_half(0), wqk_half(1)], 1),
        "wvi": np.stack([w_im_half(wa3[:, :C]), w_im_half(wa3[:, C:])], 1),
    }
    in_maps = []
    for b in range(B):
        m = {"x1f": x1f[b], "x2f": x2f[b], "im1": im1[b], "im2": im2[b]}
        m.update(weights)
        in_maps.append(m)
    return in_maps


def kernel(x1, x2, w1, w2, wa1, wa2, wa3):
    from concourse.bass_utils import run_bass_kernel_spmd

    x1 = np.asarray(x1, np.float32)
    x2 = np.asarray(x2, np.float32)
    in_maps = _host_prep(
        x1, x2,
        np.asarray(w1, np.float32), np.asarray(w2, np.float32),
        np.asarray(wa1, np.float32), np.asarray(wa2, np.float32),
        np.asarray(wa3, np.float32),
    )
    nc = _get_nc()
    res = run_bass_kernel_spmd(nc, in_maps, core_ids=list(range(8)))
    out0 = np.stack(
        [np.asarray(res.results[b]["out"], np.float32) for b in range(B)], 0
    ).reshape(B, 2 * C, H, W)
    out1 = np.concatenate([x1, x2], axis=1)
    return out0, out1


# revision 39
# speedup vs baseline: 1.1888x; 1.0247x over previous
"""Bass/Trainium2 kernel for nn_ButterflyGatingUnit (v2).

Data-parallel over batch B=8 across 8 NeuronCores (one image per core).

v2 redesign vs v1 (307us -> target ~230us):
  - im2col is built on the HOST into two DRAM tensors (one per input half),
    with the per-fill pixel ordering PHASE-GROUPED: free index within a
    12-row fill is (ty, tx, j, bx) where the output pixel is
    (12r + 3j + ty, 3bx + tx).  One big DMA per (input, fill) replaces
    ~26 small DMAs, and every 128-slice of the free dim is exactly one
    3x3 phase class = one attention tap's block positions for that fill.
  - q and k convs run in TRANSPOSED orientation: lhsT = im2col tile
    (m = 128 pixels, full PE rows), rhs = [wq|wk] weights (free = 192).
    The psum windows come out as qT/kT chunks [128 positions, 192] that
    feed the gram matmuls directly -- no DMA transposes at all.
  - Per-tap gram matrices accumulate in two persistent PSUM banks across
    all 8 fills (one start=True per bank; the bank's pending-zero covers
    the other taps' first write), so the attention setup leaves the
    critical path.
  - y1/y2 stay in SBUF as bf16 (phase-ordered; no DRAM round trip).  The
    epilogue is all-bf16 (x is staged as bf16, out is written bf16 and
    widened on the host), computed in SBUF with the phase->raster
    un-permute folded into the final add's access patterns, engine-
    balanced so it hides under attn@v.
  - v is un-permuted to a raster padded buffer at evacuation (3 strided
    copies per psum tile on Pool); attn @ v is the same dynamic-weight
    3x3 tap conv as v1.

All heavy matmuls are bf16 with fp32 PSUM accumulation; stats, softmax,
and BN coefficients are fp32.
"""

import os
import sys

sys.path.insert(0, "/opt/trn_rl_repo")

import numpy as np

import concourse.bass as bass
import concourse.mybir as mybir
import concourse.tile as tile
from concourse.masks import make_identity

F32 = mybir.dt.float32
BF16 = mybir.dt.bfloat16
AF = mybir.ActivationFunctionType
ALU = mybir.AluOpType
AX = mybir.AxisListType

B, C, H, W = 8, 96, 96, 96
HW = H * W                 # 9216
HP = H + 2                 # 98
NPIX = float(B * HW)       # BN count over (B,H,W)
EPS = 1e-5
S_ATTN = float(np.sqrt(C * 9.0))   # softmax scale sqrt(864)
NF = 8                     # fills (12 rows each)
FILL = HW // NF            # 1152 pixels per fill
NROW4 = 4                  # rows per attn psum tile (N = 384)
NG4 = H // NROW4           # 24 attn@v groups

# ---------------------------------------------------------------------------
# Workaround for this walrus build: only ONE sem wait is encodable per
# instruction. After Tile assigns waits, move extras onto fresh same-engine
# NoOps inserted right before the instruction (same engine + program order
# => identical blocking semantics).
_MAXW = 1


def _split_multiwaits(nc):
    for f in nc.m.functions:
        for bb in f.blocks:
            insts = bb.instructions
            if not any(
                i.sync_info is not None and len(i.sync_info.on_wait) > _MAXW
                for i in insts
            ):
                continue
            new = []
            for inst in insts:
                si = inst.sync_info
                if si is not None and len(si.on_wait) > _MAXW:
                    waits = list(si.on_wait)
                    keep, rest = waits[:_MAXW], waits[_MAXW:]
                    while rest:
                        nop = mybir.InstNoOp(name=f"I-waitsplit-{nc.next_id()}")
                        nop.engine = inst.engine
                        nop.sync_info = mybir.SyncInfo(
                            on_wait=rest[:_MAXW], on_update=[]
                        )
                        rest = rest[_MAXW:]
                        new.append(nop)
                    inst.sync_info = mybir.SyncInfo(
                        on_wait=keep, on_update=list(si.on_update)
                    )
                new.append(inst)
            bb.instructions = new


_orig_drain_and_barrier = tile.TileContext._drain_and_barrier


def _patched_drain_and_barrier(self, tick_clock, wait_clock):
    _orig_drain_and_barrier(self, tick_clock, wait_clock)
    _split_multiwaits(self.nc)


tile.TileContext._drain_and_barrier = _patched_drain_and_barrier
# ---------------------------------------------------------------------------


def build_nc_v2(collectives=True):
    nc = bass.Bass(num_devices=8)

    im1_d = nc.dram_tensor("im1", [128, 7, HW], BF16, kind="ExternalInput")
    im2_d = nc.dram_tensor("im2", [128, 7, HW], BF16, kind="ExternalInput")
    x1f_d = nc.dram_tensor("x1f", [C, HW], BF16, kind="ExternalInput")
    x2f_d = nc.dram_tensor("x2f", [C, HW], BF16, kind="ExternalInput")
    w1i_d = nc.dram_tensor("w1i", [128, 7, C], BF16, kind="ExternalInput")
    w2i_d = nc.dram_tensor("w2i", [128, 7, C], BF16, kind="ExternalInput")
    wqk_d = nc.dram_tensor("wqk", [128, 2, 7, 3 * C], BF16, kind="ExternalInput")
    out_d = nc.dram_tensor("out", [2 * C, HW], BF16, kind="ExternalOutput")

    with tile.TileContext(nc) as tc:
        with (
            tc.tile_pool(name="cst", bufs=1) as cst,
            tc.tile_pool(name="im", bufs=4) as im,
            tc.tile_pool(name="reuse", bufs=4) as reuse,
            tc.tile_pool(name="scr", bufs=1) as scr,
            tc.tile_pool(name="ps", bufs=4, space="PSUM") as ps,
            tc.tile_pool(name="psq", bufs=2, space="PSUM") as psq,
            tc.tile_pool(name="gram", bufs=1, space="PSUM") as gram,
            tc.tile_pool(name="dram", bufs=1, space="DRAM") as dram,
        ):
            w1i = cst.tile([128, 7, C], BF16)
            wqk = cst.tile([128, 2, 7, 3 * C], BF16)
            w2i = cst.tile([128, 7, C], BF16)

            # phase-ordered bf16 conv outputs kept in SBUF
            y1b = cst.tile([C, HW], BF16)
            y2b = cst.tile([C, HW], BF16)
            # qT/kT chunks: [pos-in-chunk, tap, fill, 0:96=q | 96:192=k]
            qkT = cst.tile([128, 9, NF, 2 * C], BF16)
            # raster padded v for the dynamic tap conv
            vpad = cst.tile([C, HP, HP], BF16)
            nc.gpsimd.memset(vpad[:, 0, :], 0.0)
            nc.gpsimd.memset(vpad[:, HP - 1, :], 0.0)
            nc.gpsimd.memset(vpad[:, 1 : HP - 1, 0], 0.0)
            nc.gpsimd.memset(vpad[:, 1 : HP - 1, HP - 1], 0.0)

            identb128 = cst.tile([128, 128], BF16)
            make_identity(nc, identb128)
            y3b = cst.tile([C, HW], BF16)
            st = cst.tile([C, 3, NG4], F32)
            stq = cst.tile([C, 3, NG4], F32)

            # persistent gram accumulators: taps 0-4 and 5-8
            gram1 = gram.tile([C, 5 * C], F32)
            gram2 = gram.tile([C, 4 * C], F32)

            def mm_norm(pt, lhs_of, IMs, fsl, halves):
                i_last = len(halves) * 7 - 1
                i = 0
                for hi in halves:
                    for k in range(7):
                        Kk = 128 if k < 6 else 96
                        nc.tensor.matmul(
                            pt[:],
                            lhs_of(hi, k)[0:Kk, :],
                            IMs[hi][0:Kk, k, fsl],
                            start=(i == 0),
                            stop=(i == i_last),
                        )
                        i += 1

            def emit_gram(rp, taps=range(9)):
                # one matmul per tap accumulating chunk rp into the
                # persistent gram banks; exactly one start per BANK (the
                # pending-zero region covers the other taps' first write).
                for t in taps:
                    if t < 5:
                        out_ap = gram1[:, t * C : (t + 1) * C]
                        first = rp == 0 and t == 0
                        last = rp == NF - 1 and t == 4
                    else:
                        out_ap = gram2[:, (t - 5) * C : (t - 4) * C]
                        first = rp == 0 and t == 5
                        last = rp == NF - 1 and t == 8
                    nc.tensor.matmul(
                        out_ap,
                        qkT[:, t, rp, C : 2 * C],
                        qkT[:, t, rp, 0:C],
                        start=first,
                        stop=last,
                        skip_group_check=True,
                    )

            def qk_windows(r, i, IMs, vsb):
                for t in range(3 * i, 3 * i + 3):
                    pqf = psq.tile([128, 3 * C], F32, tag="qk", name="qk")
                    pq = pqf[:, :]
                    mi = 0
                    for hi, IMh in enumerate(IMs):
                        for k in range(7):
                            Kk = 128 if k < 6 else 96
                            nc.tensor.matmul(
                                pq,
                                IMh[0:Kk, k, t * 128 : (t + 1) * 128],
                                wqk[0:Kk, hi, k, :],
                                start=(mi == 0),
                                stop=(mi == 13),
                            )
                            mi += 1
                    if t % 2 == 0:
                        nc.vector.tensor_copy(qkT[:, t, r, :], pq[:, 0 : 2 * C])
                        nc.scalar.activation(
                            out=vsb[:, t, :], in_=pq[:, 2 * C : 3 * C],
                            func=AF.Copy,
                        )
                    else:
                        nc.scalar.activation(
                            out=qkT[:, t, r, :], in_=pq[:, 0 : 2 * C],
                            func=AF.Copy,
                        )
                        nc.vector.tensor_copy(vsb[:, t, :], pq[:, 2 * C : 3 * C])

            def v_transposes(r, i, vsb):
                # vT windows -> raster vpad via PE transpose per tap
                vwin = vpad[:, 1 + 12 * r : 13 + 12 * r, 1 : 1 + W].rearrange(
                    "p (j ty) (bx tx) -> p ty tx j bx", j=4, ty=3, bx=32, tx=3
                )
                for t in range(3 * i, 3 * i + 3):
                    ty, tx = t // 3, t % 3
                    pvf = ps.tile([C, 384], BF16, tag="convps", name="convps")
                    pv = pvf[:, 0:128]
                    nc.tensor.transpose(pv, vsb[:, t, :], identb128[:])
                    src = pv.rearrange("p (j bx) -> p j bx", j=4)
                    if t % 2 == 0:
                        nc.vector.tensor_copy(vwin[:, ty, tx], src)
                    else:
                        nc.scalar.activation(
                            out=vwin[:, ty, tx], in_=src, func=AF.Copy
                        )

            def conv_third(r, i, IMs, vsb, qk_first=False):
                """One 384-pixel third of a fill."""
                fsl = slice(i * 384, (i + 1) * 384)
                g = 3 * r + i
                if qk_first:
                    qk_windows(r, i, IMs, vsb)
                    emit_gram(r, range(3 * i, 3 * i + 3))
                # ---- y1 (x1 only), phase-ordered evac + stats ----
                ptf = ps.tile([C, 384], F32, tag="convps", name="convps")
                pt = ptf[:, :]
                mm_norm(pt, lambda h, k: w1i[:, k, :], IMs, fsl, [0])
                if qk_first:
                    # keep fill 7's ACT queue clear so the softmax exps fire
                    # as soon as the grams close
                    nc.vector.tensor_scalar(
                        out=y1b[:, r * FILL + i * 384 : r * FILL + (i + 1) * 384],
                        in0=pt[:], scalar1=1.0, scalar2=0.0,
                        op0=ALU.mult, op1=ALU.add,
                        accum_out=st[:, 0, g : g + 1],
                    )
                else:
                    nc.scalar.activation(
                        out=y1b[:, r * FILL + i * 384 : r * FILL + (i + 1) * 384],
                        in_=pt[:], func=AF.Copy, accum_out=st[:, 0, g : g + 1],
                    )
                y1sl = y1b[:, r * FILL + i * 384 : r * FILL + (i + 1) * 384]
                sq = scr.tile([C, 384], F32, tag="sqscr", name="sq")
                if qk_first:
                    sqb = scr.tile([C, 384], BF16, tag="sqb", name="sqb")
                    nc.vector.scalar_tensor_tensor(
                        out=sqb[:], in0=y1sl, scalar=1.0, in1=y1sl,
                        op0=ALU.mult, op1=ALU.mult,
                        accum_out=stq[:, 0, g : g + 1],
                    )
                else:
                    nc.scalar.activation(
                        out=sq[:], in_=pt[:], func=AF.Square,
                        accum_out=stq[:, 0, g : g + 1],
                    )
                if not qk_first:
                    qk_windows(r, i, IMs, vsb)
                v_transposes(r, i, vsb)
                # ---- y2 (x2 only) ----
                ptf = ps.tile([C, 384], F32, tag="convps", name="convps")
                pt = ptf[:, :]
                mm_norm(pt, lambda h, k: w2i[:, k, :], IMs, fsl, [1])
                nc.vector.tensor_scalar(
                    out=y2b[:, r * FILL + i * 384 : r * FILL + (i + 1) * 384],
                    in0=pt[:], scalar1=1.0, scalar2=0.0,
                    op0=ALU.mult, op1=ALU.add,
                    accum_out=st[:, 1, g : g + 1],
                )
                y2sl = y2b[:, r * FILL + i * 384 : r * FILL + (i + 1) * 384]
                sq = scr.tile([C, 384], BF16, tag="sqscr", name="sq")
                nc.vector.scalar_tensor_tensor(
                    out=sq[:], in0=y2sl, scalar=1.0, in1=y2sl,
                    op0=ALU.mult, op1=ALU.mult,
                    accum_out=stq[:, 1, g : g + 1],
                )

            for r in range(NF):
                IM1 = im.tile([128, 7, FILL], BF16, tag="im", name="im1")
                IM2 = im.tile([128, 7, FILL], BF16, tag="im", name="im2")
                sl_r = slice(r * FILL, (r + 1) * FILL)
                if r == 0:
                    # interleave piecewise so compute starts after ~2us;
                    # later-needed weights load between the pieces
                    nc.sync.dma_start(IM1[:, :, 0:384], im1_d[:, :, 0:384])
                    nc.scalar.dma_start(w1i[:], w1i_d[:])
                    nc.sync.dma_start(IM2[:, :, 0:384], im2_d[:, :, 0:384])
                    nc.scalar.dma_start(wqk[:], wqk_d[:])
                    nc.sync.dma_start(IM1[:, :, 384:768], im1_d[:, :, 384:768])
                    nc.sync.dma_start(IM2[:, :, 384:768], im2_d[:, :, 384:768])
                    nc.scalar.dma_start(w2i[:], w2i_d[:])
                    nc.sync.dma_start(IM1[:, :, 768:FILL], im1_d[:, :, 768:FILL])
                    nc.sync.dma_start(IM2[:, :, 768:FILL], im2_d[:, :, 768:FILL])
                else:
                    nc.sync.dma_start(IM1[:], im1_d[:, :, sl_r])
                    nc.sync.dma_start(IM2[:], im2_d[:, :, sl_r])
                if r == 1:
                    # prefill out halves with x; epilogue accum-DMAs add the
                    # bn terms on top
                    nc.scalar.dma_start(out_d[0:C, :], x1f_d[:])

                # gram partials for the previous fill's chunk; the last
                # fill's own partials are interleaved with its qk windows,
                # so its predecessor must be accumulated first
                vsb = im.tile([128, 9, C], BF16, tag="vsb", bufs=2,
                              name="vsb")
                for i in range(3):
                    conv_third(r, i, [IM1, IM2], vsb, qk_first=False)
                if r > 0:
                    emit_gram(r - 1)
            emit_gram(NF - 1)

            # ------- softmax: exp of grams, denominator, transposes -------
            # No max shift: logits/sqrt(864) are O(few), safe for fp32 exp.
            A = cst.tile([C, 9, C], F32)
            attnT = cst.tile([C, 9, C], BF16)
            identf = cst.tile([C, C], F32)
            make_identity(nc, identf)
            def transpose_tap(t):
                ppf = ps.tile([C, 384], F32, tag="convps", name="convps")
                pp = ppf[:, 0:C]
                nc.tensor.transpose(pp, A[:, t, :], identf[:])
                if t % 2 == 0:
                    nc.scalar.activation(out=attnT[:, t, :], in_=pp, func=AF.Copy)
                else:
                    nc.vector.tensor_copy(attnT[:, t, :], pp)

            nc.scalar.activation(
                A[:, 0:5, :].rearrange("p a b -> p (a b)"), gram1[:, :],
                AF.Exp, scale=1.0 / S_ATTN,
            )
            for t in range(5):
                transpose_tap(t)
            nc.scalar.activation(
                A[:, 5:9, :].rearrange("p a b -> p (a b)"), gram2[:, :],
                AF.Exp, scale=1.0 / S_ATTN,
            )
            for t in range(5, 9):
                transpose_tap(t)
            Aflat = A[:].rearrange("p a b -> p (a b)")
            dsum = cst.tile([C, 1], F32)
            nc.vector.reduce_sum(dsum[:], Aflat, axis=AX.X)
            rd = cst.tile([C, 1], F32)
            nc.vector.reciprocal(rd[:], dsum[:])

            # ---------------- stats collective #1 (y1, y2) --------------
            stats1 = cst.tile([C, 4], F32)
            nc.vector.reduce_sum(stats1[:, 0:1], st[:, 0, :], axis=AX.X)
            nc.vector.reduce_sum(stats1[:, 1:2], stq[:, 0, :], axis=AX.X)
            nc.vector.reduce_sum(stats1[:, 2:3], st[:, 1, :], axis=AX.X)
            nc.vector.reduce_sum(stats1[:, 3:4], stq[:, 1, :], axis=AX.X)
            cc1_in = dram.tile([C, 4], F32)
            cc1_out = dram.tile([C, 4], F32)
            nc.sync.dma_start(cc1_in[:], stats1[:])
            if collectives:
                nc.gpsimd.collective_compute(
                    "AllReduce", ALU.add, replica_groups=[list(range(8))],
                    ins=[cc1_in[:].opt()], outs=[cc1_out[:].opt()],
                )
            else:
                nc.sync.dma_start(cc1_out[:], cc1_in[:])
            stats1r = cst.tile([C, 4], F32)
            nc.sync.dma_start(stats1r[:], cc1_out[:])

            def bn_coeffs(sum_col, sq_col, label):
                mu = cst.tile([C, 1], F32, tag=f"mu_{label}", name=f"mu_{label}")
                nc.vector.tensor_scalar_mul(mu[:], sum_col, 1.0 / NPIX)
                ex2 = cst.tile([C, 1], F32, tag=f"e2_{label}", name=f"e2_{label}")
                nc.vector.tensor_scalar_mul(ex2[:], sq_col, 1.0 / NPIX)
                var = cst.tile([C, 1], F32, tag=f"v_{label}", name=f"v_{label}")
                nc.vector.tensor_tensor(var[:], mu[:], mu[:], ALU.mult)
                nc.vector.tensor_tensor(var[:], ex2[:], var[:], ALU.subtract)
                nc.vector.tensor_scalar_add(var[:], var[:], EPS)
                sd = cst.tile([C, 1], F32, tag=f"s_{label}", name=f"s_{label}")
                nc.scalar.activation(sd[:], var[:], AF.Sqrt)
                r_ = cst.tile([C, 1], F32, tag=f"r_{label}", name=f"r_{label}")
                nc.vector.reciprocal(r_[:], sd[:])
                mb = None
                if label in ("y2", "y3"):
                    mb = cst.tile([C, 1], F32, tag=f"m_{label}",
                                  name=f"m_{label}")
                    nc.vector.tensor_scalar(
                        out=mb[:], in0=mu[:], scalar1=r_[:], scalar2=-1.0,
                        op0=ALU.mult, op1=ALU.mult,
                    )
                return mu, r_, mb

            mu1, r1, mb1 = bn_coeffs(stats1r[:, 0:1], stats1r[:, 1:2], "y1")
            mu2, r2, mb2 = bn_coeffs(stats1r[:, 2:3], stats1r[:, 3:4], "y2")

            # ------------- attn @ v with interleaved epilogue-1 ----------
            # out[0:96] = x1 + bn(y1)*bn(y2), all bf16, computed in SBUF and
            # written once; the phase->raster un-permute rides on the final
            # add's access patterns.  Engine split per chunk: t1 ACT,
            # t2 DVE, g12 Pool, un-permute adds DVE, write on SP.
            # combined scalars: gated = (y1-mu1)*(rc*y2 + bc),
            # rc = r1*r2, bc = -mu2*rc
            rc = cst.tile([C, 1], F32)
            nc.vector.tensor_tensor(rc[:], r1[:], r2[:], ALU.mult)
            bc = cst.tile([C, 1], F32)
            nc.vector.tensor_scalar(
                out=bc[:], in0=mu2[:], scalar1=rc[:], scalar2=-1.0,
                op0=ALU.mult, op1=ALU.mult,
            )

            def epi1_chunk(c0):
                sl = slice(c0 * FILL, (c0 + 1) * FILL)
                t2 = reuse.tile([C, FILL], BF16, tag="reuse")
                nc.scalar.activation(
                    t2[:], y2b[:, sl], AF.Identity, bias=bc[:], scale=rc[:]
                )
                # g12 = (y1-mu1)*t2 with the phase->raster un-permute on the
                # dst AP (split per ty to keep APs at 4 dims)
                g12 = reuse.tile([C, FILL], BF16, tag="reuse")
                # HW limits stt outputs to <=2 free dims: iterate (ty, j),
                # each piece is [p, tx, bx]
                t1v = y1b[:, sl].rearrange("p (ty tx j bx) -> p ty j tx bx",
                                           ty=3, tx=3, j=4, bx=32)
                t2v = t2[:].rearrange("p (ty tx j bx) -> p ty j tx bx",
                                      ty=3, tx=3, j=4, bx=32)
                gv = g12[:].rearrange("p (j ty bx tx) -> p ty j tx bx",
                                      j=4, ty=3, bx=32, tx=3)
                for ty in range(3):
                    for j in range(4):
                        nc.vector.scalar_tensor_tensor(
                            out=gv[:, ty, j], in0=t1v[:, ty, j], scalar=mu1[:],
                            in1=t2v[:, ty, j], op0=ALU.subtract, op1=ALU.mult,
                        )
                nc.gpsimd.dma_start(out_d[0:C, sl], g12[:], accum_op=ALU.add)

            # prefetch x2 for the tail while PE runs attn@v (im pool is
            # free after the conv phase; exactly 4 slots)
            T3CHUNKS = [(0, 1152, "dve"), (1152, 3456, "act"),
                        (3456, 5760, "dve"), (5760, 8064, "act"),
                        (8064, 9216, "dve")]
            rx2_tiles = []
            for lo, hi, eng in T3CHUNKS[:4]:
                rx2 = im.tile([C, hi - lo], BF16, tag="im", name="rx2")
                nc.scalar.dma_start(rx2[:], x2f_d[:, lo:hi])
                rx2_tiles.append(rx2)
            rx2_tiles.append(None)

            EPI1_AT = {2: 0, 5: 1, 8: 2, 10: 3, 13: 4, 15: 5, 18: 6, 20: 7}
            for g in range(NG4):
                ptf = ps.tile([C, 384], F32, tag="convps", name="convps")
                pt = ptf[:, :]
                for t in range(9):
                    ky, kx = t // 3, t % 3
                    rhs = vpad[:, NROW4 * g + ky : NROW4 * g + ky + NROW4,
                               kx : kx + W]
                    nc.tensor.matmul(
                        pt[:], attnT[:, t, :], rhs, start=(t == 0), stop=(t == 8)
                    )
                sl = slice(g * NROW4 * W, (g + 1) * NROW4 * W)
                nc.vector.tensor_scalar(
                    out=y3b[:, sl], in0=pt[:], scalar1=rd[:], scalar2=0.0,
                    op0=ALU.mult, op1=ALU.add,
                    accum_out=st[:, 2, g : g + 1],
                )
                sq = scr.tile([C, 384], F32, tag="sqscr", name="sq")
                nc.scalar.activation(
                    out=sq[:], in_=pt[:], func=AF.Square, scale=rd[:],
                    accum_out=stq[:, 2, g : g + 1],
                )
                if g in EPI1_AT:
                    epi1_chunk(EPI1_AT[g])

            # ---------------- stats collective #2 (y3) ----------------
            stats2 = cst.tile([C, 2], F32)
            nc.vector.reduce_sum(stats2[:, 0:1], st[:, 2, :], axis=AX.X)
            nc.vector.reduce_sum(stats2[:, 1:2], stq[:, 2, :], axis=AX.X)
            cc2_in = dram.tile([C, 2], F32)
            cc2_out = dram.tile([C, 2], F32)
            nc.sync.dma_start(cc2_in[:], stats2[:])
            if collectives:
                nc.gpsimd.collective_compute(
                    "AllReduce", ALU.add, replica_groups=[list(range(8))],
                    ins=[cc2_in[:].opt()], outs=[cc2_out[:].opt()],
                )
            else:
                nc.sync.dma_start(cc2_out[:], cc2_in[:])
            stats2r = cst.tile([C, 2], F32)
            nc.sync.dma_start(stats2r[:], cc2_out[:])
            mu3, r3, mb3 = bn_coeffs(stats2r[:, 0:1], stats2r[:, 1:2], "y3")

            # ---------------- epilogue half 2: x2 + bn(y3) ----------------
            # t3 rotates ACT/DVE/Pool, adds rotate DVE/ACT? (ACT cannot add
            # two tensors) -> adds on DVE (bf16 2x) and Pool alternating.
            for (lo, hi, eng), rx2 in zip(T3CHUNKS, rx2_tiles):
                sl = slice(lo, hi)
                if rx2 is None:
                    rx2 = im.tile([C, hi - lo], BF16, tag="im", name="rx2")
                    nc.sync.dma_start(rx2[:], x2f_d[:, sl])
                t3 = reuse.tile([C, hi - lo], BF16, tag="t3", bufs=3,
                                padded_shape=[C, 2304], name="t3")
                if eng == "dve":
                    nc.vector.tensor_scalar(
                        out=t3[:], in0=y3b[:, sl], scalar1=mu3[:], scalar2=r3[:],
                        op0=ALU.subtract, op1=ALU.mult,
                    )
                else:
                    nc.scalar.activation(
                        t3[:], y3b[:, sl], AF.Identity, bias=mb3[:], scale=r3[:]
                    )
                nc.vector.tensor_tensor(t3[:], t3[:], rx2[:], ALU.add)
                nc.sync.dma_start(out_d[C : 2 * C, sl], t3[:])

    return nc


# alias for profiling scripts
build_nc = build_nc_v2

_CACHED_NC = None


def _get_nc():
    global _CACHED_NC
    if _CACHED_NC is None:
        _CACHED_NC = build_nc_v2()
    return _CACHED_NC


def _host_prep(x1, x2, w1, w2, wa1, wa2, wa3):
    import ml_dtypes

    bf = ml_dtypes.bfloat16

    x1f = np.ascontiguousarray(x1.reshape(B, C, HW)).astype(bf)
    x2f = np.ascontiguousarray(x2.reshape(B, C, HW)).astype(bf)

    def im2col_phase(x):
        # x: [B, C, H, W] fp32 -> [B, 128, 7, HW] bf16 im2col with rows
        # u = t*96 + ci (t = 3x3 conv tap, raster) packed as u = k*128 + p,
        # and free dim per 12-row fill ordered (ty, tx, j, bx) for output
        # pixel (12r + 3j + ty, 3bx + tx).
        xp = np.zeros((B, C, H + 2, W + 2), bf)
        xp[:, :, 1 : 1 + H, 1 : 1 + W] = x.astype(bf)
        out = np.zeros((B, 896, NF, 3, 3, 4, 32), bf)
        for ky in range(3):
            for kx in range(3):
                t = ky * 3 + kx
                v = xp[:, :, ky : ky + H, kx : kx + W]
                # rows: y = 12r + 3j + ty -> (r, j, ty); cols: x = 3bx+tx
                v = v.reshape(B, C, NF, 4, 3, 32, 3)
                # [B, ci, r, j, ty, bx, tx] -> [B, ci, r, ty, tx, j, bx]
                out[:, t * C : (t + 1) * C] = v.transpose(0, 1, 2, 4, 6, 3, 5)
        out = out.reshape(B, 7, 128, HW).transpose(0, 2, 1, 3)
        return np.ascontiguousarray(out)

    im1 = im2col_phase(x1)
    im2 = im2col_phase(x2)

    def w_im_half(w):
        # [Cout, 96, 3, 3] -> [128, 7, Cout] with row u = t*96+ci (padded)
        co, ci = w.shape[0], w.shape[1]
        u = np.transpose(w.reshape(co, ci, 9), (2, 1, 0)).reshape(9 * ci, co)
        up = np.zeros((896, co), np.float32)
        up[: 9 * ci] = u
        return np.ascontiguousarray(
            up.reshape(7, 128, co).transpose(1, 0, 2)
        ).astype(bf)

    def wqk_half(h):
        sl = slice(0, C) if h == 0 else slice(C, 2 * C)
        return np.concatenate([w_im_half(wa1[:, sl]), w_im_half(wa2[:, sl]),
                               w_im_half(wa3[:, sl])], axis=-1)

    weights = {
        "w1i": w_im_half(w1),
        "w2i": w_im_half(w2),
        "wqk": np.stack([wqk_half(0), wqk_half(1)], 1),
    }
    in_maps = []
    for b in range(B):
        m = {"x1f": x1f[b], "x2f": x2f[b], "im1": im1[b], "im2": im2[b]}
        m.update(weights)
        in_maps.append(m)
    return in_maps


def kernel(x1, x2, w1, w2, wa1, wa2, wa3):
    from concourse.bass_utils import run_bass_kernel_spmd

    x1 = np.asarray(x1, np.float32)
    x2 = np.asarray(x2, np.float32)
    in_maps = _host_prep(
        x1, x2,
        np.asarray(w1, np.float32), np.asarray(w2, np.float32),
        np.asarray(wa1, np.float32), np.asarray(wa2, np.float32),
        np.asarray(wa3, np.float32),
    )
    nc = _get_nc()
    res = run_bass_kernel_spmd(nc, in_maps, core_ids=list(range(8)))
    out0 = np.stack(
        [np.asarray(res.results[b]["out"], np.float32) for b in range(B)], 0
    ).reshape(B, 2 * C, H, W)
    out1 = np.concatenate([x1, x2], axis=1)
    return out0, out1


# revision 40
# speedup vs baseline: 1.1993x; 1.0088x over previous
"""Bass/Trainium2 kernel for nn_ButterflyGatingUnit (v2).

Data-parallel over batch B=8 across 8 NeuronCores (one image per core).

v2 redesign vs v1 (307us -> target ~230us):
  - im2col is built on the HOST into two DRAM tensors (one per input half),
    with the per-fill pixel ordering PHASE-GROUPED: free index within a
    12-row fill is (ty, tx, j, bx) where the output pixel is
    (12r + 3j + ty, 3bx + tx).  One big DMA per (input, fill) replaces
    ~26 small DMAs, and every 128-slice of the free dim is exactly one
    3x3 phase class = one attention tap's block positions for that fill.
  - q and k convs run in TRANSPOSED orientation: lhsT = im2col tile
    (m = 128 pixels, full PE rows), rhs = [wq|wk] weights (free = 192).
    The psum windows come out as qT/kT chunks [128 positions, 192] that
    feed the gram matmuls directly -- no DMA transposes at all.
  - Per-tap gram matrices accumulate in two persistent PSUM banks across
    all 8 fills (one start=True per bank; the bank's pending-zero covers
    the other taps' first write), so the attention setup leaves the
    critical path.
  - y1/y2 stay in SBUF as bf16 (phase-ordered; no DRAM round trip).  The
    epilogue is all-bf16 (x is staged as bf16, out is written bf16 and
    widened on the host), computed in SBUF with the phase->raster
    un-permute folded into the final add's access patterns, engine-
    balanced so it hides under attn@v.
  - v is un-permuted to a raster padded buffer at evacuation (3 strided
    copies per psum tile on Pool); attn @ v is the same dynamic-weight
    3x3 tap conv as v1.

All heavy matmuls are bf16 with fp32 PSUM accumulation; stats, softmax,
and BN coefficients are fp32.
"""

import os
import sys

sys.path.insert(0, "/opt/trn_rl_repo")

import numpy as np

import concourse.bass as bass
import concourse.mybir as mybir
import concourse.tile as tile
from concourse.masks import make_identity

F32 = mybir.dt.float32
BF16 = mybir.dt.bfloat16
AF = mybir.ActivationFunctionType
ALU = mybir.AluOpType
AX = mybir.AxisListType

B, C, H, W = 8, 96, 96, 96
HW = H * W                 # 9216
HP = H + 2                 # 98
NPIX = float(B * HW)       # BN count over (B,H,W)
EPS = 1e-5
S_ATTN = float(np.sqrt(C * 9.0))   # softmax scale sqrt(864)
NF = 8                     # fills (12 rows each)
FILL = HW // NF            # 1152 pixels per fill
NROW4 = 4                  # rows per attn psum tile (N = 384)
NG4 = H // NROW4           # 24 attn@v groups

# ---------------------------------------------------------------------------
# Workaround for this walrus build: only ONE sem wait is encodable per
# instruction. After Tile assigns waits, move extras onto fresh same-engine
# NoOps inserted right before the instruction (same engine + program order
# => identical blocking semantics).
_MAXW = 1


def _split_multiwaits(nc):
    for f in nc.m.functions:
        for bb in f.blocks:
            insts = bb.instructions
            if not any(
                i.sync_info is not None and len(i.sync_info.on_wait) > _MAXW
                for i in insts
            ):
                continue
            new = []
            for inst in insts:
                si = inst.sync_info
                if si is not None and len(si.on_wait) > _MAXW:
                    waits = list(si.on_wait)
                    keep, rest = waits[:_MAXW], waits[_MAXW:]
                    while rest:
                        nop = mybir.InstNoOp(name=f"I-waitsplit-{nc.next_id()}")
                        nop.engine = inst.engine
                        nop.sync_info = mybir.SyncInfo(
                            on_wait=rest[:_MAXW], on_update=[]
                        )
                        rest = rest[_MAXW:]
                        new.append(nop)
                    inst.sync_info = mybir.SyncInfo(
                        on_wait=keep, on_update=list(si.on_update)
                    )
                new.append(inst)
            bb.instructions = new


_orig_drain_and_barrier = tile.TileContext._drain_and_barrier


def _patched_drain_and_barrier(self, tick_clock, wait_clock):
    _orig_drain_and_barrier(self, tick_clock, wait_clock)
    _split_multiwaits(self.nc)


tile.TileContext._drain_and_barrier = _patched_drain_and_barrier
# ---------------------------------------------------------------------------


def build_nc_v2(collectives=True):
    nc = bass.Bass(num_devices=8)

    im1_d = nc.dram_tensor("im1", [128, 7, HW], BF16, kind="ExternalInput")
    im2_d = nc.dram_tensor("im2", [128, 7, HW], BF16, kind="ExternalInput")
    x1f_d = nc.dram_tensor("x1f", [C, HW], BF16, kind="ExternalInput")
    x2f_d = nc.dram_tensor("x2f", [C, HW], BF16, kind="ExternalInput")
    w1i_d = nc.dram_tensor("w1i", [128, 7, C], BF16, kind="ExternalInput")
    w2i_d = nc.dram_tensor("w2i", [128, 7, C], BF16, kind="ExternalInput")
    wqk_d = nc.dram_tensor("wqk", [128, 2, 7, 3 * C], BF16, kind="ExternalInput")
    out_d = nc.dram_tensor("out", [2 * C, HW], BF16, kind="ExternalOutput")

    with tile.TileContext(nc) as tc:
        with (
            tc.tile_pool(name="cst", bufs=1) as cst,
            tc.tile_pool(name="im", bufs=4) as im,
            tc.tile_pool(name="reuse", bufs=4) as reuse,
            tc.tile_pool(name="scr", bufs=1) as scr,
            tc.tile_pool(name="ps", bufs=4, space="PSUM") as ps,
            tc.tile_pool(name="psq", bufs=2, space="PSUM") as psq,
            tc.tile_pool(name="gram", bufs=1, space="PSUM") as gram,
            tc.tile_pool(name="dram", bufs=1, space="DRAM") as dram,
        ):
            w1i = cst.tile([128, 7, C], BF16)
            wqk = cst.tile([128, 2, 7, 3 * C], BF16)
            w2i = cst.tile([128, 7, C], BF16)

            # phase-ordered bf16 conv outputs kept in SBUF
            y1b = cst.tile([C, HW], BF16)
            y2b = cst.tile([C, HW], BF16)
            # qT/kT chunks: [pos-in-chunk, tap, fill, 0:96=q | 96:192=k]
            qkT = cst.tile([128, 9, NF, 2 * C], BF16)
            # raster padded v for the dynamic tap conv
            vpad = cst.tile([C, HP, HP], BF16)
            nc.gpsimd.memset(vpad[:, 0, :], 0.0)
            nc.gpsimd.memset(vpad[:, HP - 1, :], 0.0)
            nc.gpsimd.memset(vpad[:, 1 : HP - 1, 0], 0.0)
            nc.gpsimd.memset(vpad[:, 1 : HP - 1, HP - 1], 0.0)

            identb128 = cst.tile([128, 128], BF16)
            make_identity(nc, identb128)
            y3b = cst.tile([C, HW], BF16)
            st = cst.tile([C, 3, NG4], F32)
            stq = cst.tile([C, 3, NG4], F32)

            # persistent gram accumulators: taps 0-4 and 5-8
            gram1 = gram.tile([C, 5 * C], F32)
            gram2 = gram.tile([C, 4 * C], F32)

            def mm_norm(pt, lhs_of, IMs, fsl, halves):
                i_last = len(halves) * 7 - 1
                i = 0
                for hi in halves:
                    for k in range(7):
                        Kk = 128 if k < 6 else 96
                        nc.tensor.matmul(
                            pt[:],
                            lhs_of(hi, k)[0:Kk, :],
                            IMs[hi][0:Kk, k, fsl],
                            start=(i == 0),
                            stop=(i == i_last),
                        )
                        i += 1

            def emit_gram(rp, taps=range(9)):
                # one matmul per tap accumulating chunk rp into the
                # persistent gram banks; exactly one start per BANK (the
                # pending-zero region covers the other taps' first write).
                for t in taps:
                    if t < 5:
                        out_ap = gram1[:, t * C : (t + 1) * C]
                        first = rp == 0 and t == 0
                        last = rp == NF - 1 and t == 4
                    else:
                        out_ap = gram2[:, (t - 5) * C : (t - 4) * C]
                        first = rp == 0 and t == 5
                        last = rp == NF - 1 and t == 8
                    nc.tensor.matmul(
                        out_ap,
                        qkT[:, t, rp, C : 2 * C],
                        qkT[:, t, rp, 0:C],
                        start=first,
                        stop=last,
                        skip_group_check=True,
                    )

            def qk_windows(r, i, IMs, vsb):
                for t in range(3 * i, 3 * i + 3):
                    pqf = psq.tile([128, 3 * C], F32, tag="qk", name="qk")
                    pq = pqf[:, :]
                    mi = 0
                    for hi, IMh in enumerate(IMs):
                        for k in range(7):
                            Kk = 128 if k < 6 else 96
                            nc.tensor.matmul(
                                pq,
                                IMh[0:Kk, k, t * 128 : (t + 1) * 128],
                                wqk[0:Kk, hi, k, :],
                                start=(mi == 0),
                                stop=(mi == 13),
                            )
                            mi += 1
                    if t % 2 == 0:
                        nc.vector.tensor_copy(qkT[:, t, r, :], pq[:, 0 : 2 * C])
                        nc.scalar.activation(
                            out=vsb[:, t, :], in_=pq[:, 2 * C : 3 * C],
                            func=AF.Copy,
                        )
                    else:
                        nc.scalar.activation(
                            out=qkT[:, t, r, :], in_=pq[:, 0 : 2 * C],
                            func=AF.Copy,
                        )
                        nc.vector.tensor_copy(vsb[:, t, :], pq[:, 2 * C : 3 * C])

            def v_transposes(r, i, vsb):
                # vT windows -> raster vpad via PE transpose per tap
                vwin = vpad[:, 1 + 12 * r : 13 + 12 * r, 1 : 1 + W].rearrange(
                    "p (j ty) (bx tx) -> p ty tx j bx", j=4, ty=3, bx=32, tx=3
                )
                for t in range(3 * i, 3 * i + 3):
                    ty, tx = t // 3, t % 3
                    pvf = ps.tile([C, 384], BF16, tag="convps", name="convps")
                    pv = pvf[:, 0:128]
                    nc.tensor.transpose(pv, vsb[:, t, :], identb128[:])
                    src = pv.rearrange("p (j bx) -> p j bx", j=4)
                    if t % 2 == 0:
                        nc.vector.tensor_copy(vwin[:, ty, tx], src)
                    else:
                        nc.scalar.activation(
                            out=vwin[:, ty, tx], in_=src, func=AF.Copy
                        )

            def conv_third(r, i, IMs, vsb, qk_first=False):
                """One 384-pixel third of a fill."""
                fsl = slice(i * 384, (i + 1) * 384)
                g = 3 * r + i
                if qk_first:
                    qk_windows(r, i, IMs, vsb)
                    emit_gram(r, range(3 * i, 3 * i + 3))
                # ---- y1 (x1 only), phase-ordered evac + stats ----
                ptf = ps.tile([C, 384], F32, tag="convps", name="convps")
                pt = ptf[:, :]
                mm_norm(pt, lambda h, k: w1i[:, k, :], IMs, fsl, [0])
                if qk_first:
                    # keep fill 7's ACT queue clear so the softmax exps fire
                    # as soon as the grams close
                    nc.vector.tensor_scalar(
                        out=y1b[:, r * FILL + i * 384 : r * FILL + (i + 1) * 384],
                        in0=pt[:], scalar1=1.0, scalar2=0.0,
                        op0=ALU.mult, op1=ALU.add,
                        accum_out=st[:, 0, g : g + 1],
                    )
                else:
                    nc.scalar.activation(
                        out=y1b[:, r * FILL + i * 384 : r * FILL + (i + 1) * 384],
                        in_=pt[:], func=AF.Copy, accum_out=st[:, 0, g : g + 1],
                    )
                y1sl = y1b[:, r * FILL + i * 384 : r * FILL + (i + 1) * 384]
                sq = scr.tile([C, 384], F32, tag="sqscr", name="sq")
                if qk_first:
                    sqb = scr.tile([C, 384], BF16, tag="sqb", name="sqb")
                    nc.vector.scalar_tensor_tensor(
                        out=sqb[:], in0=y1sl, scalar=1.0, in1=y1sl,
                        op0=ALU.mult, op1=ALU.mult,
                        accum_out=stq[:, 0, g : g + 1],
                    )
                else:
                    nc.scalar.activation(
                        out=sq[:], in_=pt[:], func=AF.Square,
                        accum_out=stq[:, 0, g : g + 1],
                    )
                if not qk_first:
                    qk_windows(r, i, IMs, vsb)
                if i > 0:
                    v_transposes(r, i - 1, vsb)
                # ---- y2 (x2 only) ----
                ptf = ps.tile([C, 384], F32, tag="convps", name="convps")
                pt = ptf[:, :]
                mm_norm(pt, lambda h, k: w2i[:, k, :], IMs, fsl, [1])
                nc.vector.tensor_scalar(
                    out=y2b[:, r * FILL + i * 384 : r * FILL + (i + 1) * 384],
                    in0=pt[:], scalar1=1.0, scalar2=0.0,
                    op0=ALU.mult, op1=ALU.add,
                    accum_out=st[:, 1, g : g + 1],
                )
                y2sl = y2b[:, r * FILL + i * 384 : r * FILL + (i + 1) * 384]
                sq = scr.tile([C, 384], BF16, tag="sqscr", name="sq")
                nc.vector.scalar_tensor_tensor(
                    out=sq[:], in0=y2sl, scalar=1.0, in1=y2sl,
                    op0=ALU.mult, op1=ALU.mult,
                    accum_out=stq[:, 1, g : g + 1],
                )

            for r in range(NF):
                IM1 = im.tile([128, 7, FILL], BF16, tag="im", name="im1")
                IM2 = im.tile([128, 7, FILL], BF16, tag="im", name="im2")
                sl_r = slice(r * FILL, (r + 1) * FILL)
                if r == 0:
                    # interleave piecewise so compute starts after ~2us;
                    # later-needed weights load between the pieces
                    nc.sync.dma_start(IM1[:, :, 0:384], im1_d[:, :, 0:384])
                    nc.scalar.dma_start(w1i[:], w1i_d[:])
                    nc.sync.dma_start(IM2[:, :, 0:384], im2_d[:, :, 0:384])
                    nc.scalar.dma_start(wqk[:], wqk_d[:])
                    nc.sync.dma_start(IM1[:, :, 384:768], im1_d[:, :, 384:768])
                    nc.sync.dma_start(IM2[:, :, 384:768], im2_d[:, :, 384:768])
                    nc.scalar.dma_start(w2i[:], w2i_d[:])
                    nc.sync.dma_start(IM1[:, :, 768:FILL], im1_d[:, :, 768:FILL])
                    nc.sync.dma_start(IM2[:, :, 768:FILL], im2_d[:, :, 768:FILL])
                else:
                    nc.sync.dma_start(IM1[:], im1_d[:, :, sl_r])
                    nc.sync.dma_start(IM2[:], im2_d[:, :, sl_r])
                if r == 1:
                    # prefill out halves with x; epilogue accum-DMAs add the
                    # bn terms on top
                    nc.scalar.dma_start(out_d[0:C, :], x1f_d[:])

                # gram partials for the previous fill's chunk; the last
                # fill's own partials are interleaved with its qk windows,
                # so its predecessor must be accumulated first
                vsb = im.tile([128, 9, C], BF16, tag="vsb", bufs=2,
                              name="vsb")
                for i in range(3):
                    conv_third(r, i, [IM1, IM2], vsb, qk_first=False)
                v_transposes(r, 2, vsb)
                if r > 0:
                    emit_gram(r - 1)
            emit_gram(NF - 1)

            # ------- softmax: exp of grams, denominator, transposes -------
            # No max shift: logits/sqrt(864) are O(few), safe for fp32 exp.
            A = cst.tile([C, 9, C], F32)
            attnT = cst.tile([C, 9, C], BF16)
            identf = cst.tile([C, C], F32)
            make_identity(nc, identf)
            def transpose_tap(t):
                ppf = ps.tile([C, 384], F32, tag="convps", name="convps")
                pp = ppf[:, 0:C]
                nc.tensor.transpose(pp, A[:, t, :], identf[:])
                if t % 2 == 0:
                    nc.scalar.activation(out=attnT[:, t, :], in_=pp, func=AF.Copy)
                else:
                    nc.vector.tensor_copy(attnT[:, t, :], pp)

            nc.scalar.activation(
                A[:, 0:5, :].rearrange("p a b -> p (a b)"), gram1[:, :],
                AF.Exp, scale=1.0 / S_ATTN,
            )
            for t in range(5):
                transpose_tap(t)
            nc.scalar.activation(
                A[:, 5:9, :].rearrange("p a b -> p (a b)"), gram2[:, :],
                AF.Exp, scale=1.0 / S_ATTN,
            )
            for t in range(5, 9):
                transpose_tap(t)
            Aflat = A[:].rearrange("p a b -> p (a b)")
            dsum = cst.tile([C, 1], F32)
            nc.vector.reduce_sum(dsum[:], Aflat, axis=AX.X)
            rd = cst.tile([C, 1], F32)
            nc.vector.reciprocal(rd[:], dsum[:])

            # ---------------- stats collective #1 (y1, y2) --------------
            stats1 = cst.tile([C, 4], F32)
            nc.vector.reduce_sum(stats1[:, 0:1], st[:, 0, :], axis=AX.X)
            nc.vector.reduce_sum(stats1[:, 1:2], stq[:, 0, :], axis=AX.X)
            nc.vector.reduce_sum(stats1[:, 2:3], st[:, 1, :], axis=AX.X)
            nc.vector.reduce_sum(stats1[:, 3:4], stq[:, 1, :], axis=AX.X)
            cc1_in = dram.tile([C, 4], F32)
            cc1_out = dram.tile([C, 4], F32)
            nc.sync.dma_start(cc1_in[:], stats1[:])
            if collectives:
                nc.gpsimd.collective_compute(
                    "AllReduce", ALU.add, replica_groups=[list(range(8))],
                    ins=[cc1_in[:].opt()], outs=[cc1_out[:].opt()],
                )
            else:
                nc.sync.dma_start(cc1_out[:], cc1_in[:])
            stats1r = cst.tile([C, 4], F32)
            nc.sync.dma_start(stats1r[:], cc1_out[:])

            def bn_coeffs(sum_col, sq_col, label):
                mu = cst.tile([C, 1], F32, tag=f"mu_{label}", name=f"mu_{label}")
                nc.vector.tensor_scalar_mul(mu[:], sum_col, 1.0 / NPIX)
                ex2 = cst.tile([C, 1], F32, tag=f"e2_{label}", name=f"e2_{label}")
                nc.vector.tensor_scalar_mul(ex2[:], sq_col, 1.0 / NPIX)
                var = cst.tile([C, 1], F32, tag=f"v_{label}", name=f"v_{label}")
                nc.vector.tensor_tensor(var[:], mu[:], mu[:], ALU.mult)
                nc.vector.tensor_tensor(var[:], ex2[:], var[:], ALU.subtract)
                nc.vector.tensor_scalar_add(var[:], var[:], EPS)
                sd = cst.tile([C, 1], F32, tag=f"s_{label}", name=f"s_{label}")
                nc.scalar.activation(sd[:], var[:], AF.Sqrt)
                r_ = cst.tile([C, 1], F32, tag=f"r_{label}", name=f"r_{label}")
                nc.vector.reciprocal(r_[:], sd[:])
                mb = None
                if label in ("y2", "y3"):
                    mb = cst.tile([C, 1], F32, tag=f"m_{label}",
                                  name=f"m_{label}")
                    nc.vector.tensor_scalar(
                        out=mb[:], in0=mu[:], scalar1=r_[:], scalar2=-1.0,
                        op0=ALU.mult, op1=ALU.mult,
                    )
                return mu, r_, mb

            mu1, r1, mb1 = bn_coeffs(stats1r[:, 0:1], stats1r[:, 1:2], "y1")
            mu2, r2, mb2 = bn_coeffs(stats1r[:, 2:3], stats1r[:, 3:4], "y2")

            # ------------- attn @ v with interleaved epilogue-1 ----------
            # out[0:96] = x1 + bn(y1)*bn(y2), all bf16, computed in SBUF and
            # written once; the phase->raster un-permute rides on the final
            # add's access patterns.  Engine split per chunk: t1 ACT,
            # t2 DVE, g12 Pool, un-permute adds DVE, write on SP.
            # combined scalars: gated = (y1-mu1)*(rc*y2 + bc),
            # rc = r1*r2, bc = -mu2*rc
            rc = cst.tile([C, 1], F32)
            nc.vector.tensor_tensor(rc[:], r1[:], r2[:], ALU.mult)
            bc = cst.tile([C, 1], F32)
            nc.vector.tensor_scalar(
                out=bc[:], in0=mu2[:], scalar1=rc[:], scalar2=-1.0,
                op0=ALU.mult, op1=ALU.mult,
            )

            def epi1_chunk(c0):
                sl = slice(c0 * FILL, (c0 + 1) * FILL)
                t2 = reuse.tile([C, FILL], BF16, tag="reuse")
                nc.scalar.activation(
                    t2[:], y2b[:, sl], AF.Identity, bias=bc[:], scale=rc[:]
                )
                # g12 = (y1-mu1)*t2 with the phase->raster un-permute on the
                # dst AP (split per ty to keep APs at 4 dims)
                g12 = reuse.tile([C, FILL], BF16, tag="reuse")
                # HW limits stt outputs to <=2 free dims: iterate (ty, j),
                # each piece is [p, tx, bx]
                t1v = y1b[:, sl].rearrange("p (ty tx j bx) -> p ty j tx bx",
                                           ty=3, tx=3, j=4, bx=32)
                t2v = t2[:].rearrange("p (ty tx j bx) -> p ty j tx bx",
                                      ty=3, tx=3, j=4, bx=32)
                gv = g12[:].rearrange("p (j ty bx tx) -> p ty j tx bx",
                                      j=4, ty=3, bx=32, tx=3)
                for ty in range(3):
                    for j in range(4):
                        nc.vector.scalar_tensor_tensor(
                            out=gv[:, ty, j], in0=t1v[:, ty, j], scalar=mu1[:],
                            in1=t2v[:, ty, j], op0=ALU.subtract, op1=ALU.mult,
                        )
                nc.gpsimd.dma_start(out_d[0:C, sl], g12[:], accum_op=ALU.add)

            # prefetch x2 for the tail while PE runs attn@v (im pool is
            # free after the conv phase; exactly 4 slots)
            T3CHUNKS = [(0, 1152, "dve"), (1152, 3456, "act"),
                        (3456, 5760, "dve"), (5760, 8064, "act"),
                        (8064, 9216, "dve")]
            rx2_tiles = []
            for lo, hi, eng in T3CHUNKS[:4]:
                rx2 = im.tile([C, hi - lo], BF16, tag="im", name="rx2")
                nc.scalar.dma_start(rx2[:], x2f_d[:, lo:hi])
                rx2_tiles.append(rx2)
            rx2_tiles.append(None)

            EPI1_AT = {2: 0, 5: 1, 8: 2, 10: 3, 13: 4, 15: 5, 18: 6, 20: 7}
            for g in range(NG4):
                ptf = ps.tile([C, 384], F32, tag="convps", name="convps")
                pt = ptf[:, :]
                for t in range(9):
                    ky, kx = t // 3, t % 3
                    rhs = vpad[:, NROW4 * g + ky : NROW4 * g + ky + NROW4,
                               kx : kx + W]
                    nc.tensor.matmul(
                        pt[:], attnT[:, t, :], rhs, start=(t == 0), stop=(t == 8)
                    )
                sl = slice(g * NROW4 * W, (g + 1) * NROW4 * W)
                nc.vector.tensor_scalar(
                    out=y3b[:, sl], in0=pt[:], scalar1=rd[:], scalar2=0.0,
                    op0=ALU.mult, op1=ALU.add,
                    accum_out=st[:, 2, g : g + 1],
                )
                sq = scr.tile([C, 384], F32, tag="sqscr", name="sq")
                nc.scalar.activation(
                    out=sq[:], in_=pt[:], func=AF.Square, scale=rd[:],
                    accum_out=stq[:, 2, g : g + 1],
                )
                if g in EPI1_AT:
                    epi1_chunk(EPI1_AT[g])

            # ---------------- stats collective #2 (y3) ----------------
            stats2 = cst.tile([C, 2], F32)
            nc.vector.reduce_sum(stats2[:, 0:1], st[:, 2, :], axis=AX.X)
            nc.vector.reduce_sum(stats2[:, 1:2], stq[:, 2, :], axis=AX.X)
            cc2_in = dram.tile([C, 2], F32)
            cc2_out = dram.tile([C, 2], F32)
            nc.sync.dma_start(cc2_in[:], stats2[:])
            if collectives:
                nc.gpsimd.collective_compute(
                    "AllReduce", ALU.add, replica_groups=[list(range(8))],
                    ins=[cc2_in[:].opt()], outs=[cc2_out[:].opt()],
                )
            else:
                nc.sync.dma_start(cc2_out[:], cc2_in[:])
            stats2r = cst.tile([C, 2], F32)
            nc.sync.dma_start(stats2r[:], cc2_out[:])
            mu3, r3, mb3 = bn_coeffs(stats2r[:, 0:1], stats2r[:, 1:2], "y3")

            # ---------------- epilogue half 2: x2 + bn(y3) ----------------
            # t3 rotates ACT/DVE/Pool, adds rotate DVE/ACT? (ACT cannot add
            # two tensors) -> adds on DVE (bf16 2x) and Pool alternating.
            for (lo, hi, eng), rx2 in zip(T3CHUNKS, rx2_tiles):
                sl = slice(lo, hi)
                if rx2 is None:
                    rx2 = im.tile([C, hi - lo], BF16, tag="im", name="rx2")
                    nc.sync.dma_start(rx2[:], x2f_d[:, sl])
                t3 = reuse.tile([C, hi - lo], BF16, tag="t3", bufs=3,
                                padded_shape=[C, 2304], name="t3")
                if eng == "dve":
                    nc.vector.tensor_scalar(
                        out=t3[:], in0=y3b[:, sl], scalar1=mu3[:], scalar2=r3[:],
                        op0=ALU.subtract, op1=ALU.mult,
                    )
                else:
                    nc.scalar.activation(
                        t3[:], y3b[:, sl], AF.Identity, bias=mb3[:], scale=r3[:]
                    )
                nc.vector.tensor_tensor(t3[:], t3[:], rx2[:], ALU.add)
                nc.sync.dma_start(out_d[C : 2 * C, sl], t3[:])

    return nc


# alias for profiling scripts
build_nc = build_nc_v2

_CACHED_NC = None


def _get_nc():
    global _CACHED_NC
    if _CACHED_NC is None:
        _CACHED_NC = build_nc_v2()
    return _CACHED_NC


def _host_prep(x1, x2, w1, w2, wa1, wa2, wa3):
    import ml_dtypes

    bf = ml_dtypes.bfloat16

    x1f = np.ascontiguousarray(x1.reshape(B, C, HW)).astype(bf)
    x2f = np.ascontiguousarray(x2.reshape(B, C, HW)).astype(bf)

    def im2col_phase(x):
        # x: [B, C, H, W] fp32 -> [B, 128, 7, HW] bf16 im2col with rows
        # u = t*96 + ci (t = 3x3 conv tap, raster) packed as u = k*128 + p,
        # and free dim per 12-row fill ordered (ty, tx, j, bx) for output
        # pixel (12r + 3j + ty, 3bx + tx).
        xp = np.zeros((B, C, H + 2, W + 2), bf)
        xp[:, :, 1 : 1 + H, 1 : 1 + W] = x.astype(bf)
        out = np.zeros((B, 896, NF, 3, 3, 4, 32), bf)
        for ky in range(3):
            for kx in range(3):
                t = ky * 3 + kx
                v = xp[:, :, ky : ky + H, kx : kx + W]
                # rows: y = 12r + 3j + ty -> (r, j, ty); cols: x = 3bx+tx
                v = v.reshape(B, C, NF, 4, 3, 32, 3)
                # [B, ci, r, j, ty, bx, tx] -> [B, ci, r, ty, tx, j, bx]
                out[:, t * C : (t + 1) * C] = v.transpose(0, 1, 2, 4, 6, 3, 5)
        out = out.reshape(B, 7, 128, HW).transpose(0, 2, 1, 3)
        return np.ascontiguousarray(out)

    im1 = im2col_phase(x1)
    im2 = im2col_phase(x2)

    def w_im_half(w):
        # [Cout, 96, 3, 3] -> [128, 7, Cout] with row u = t*96+ci (padded)
        co, ci = w.shape[0], w.shape[1]
        u = np.transpose(w.reshape(co, ci, 9), (2, 1, 0)).reshape(9 * ci, co)
        up = np.zeros((896, co), np.float32)
        up[: 9 * ci] = u
        return np.ascontiguousarray(
            up.reshape(7, 128, co).transpose(1, 0, 2)
        ).astype(bf)

    def wqk_half(h):
        sl = slice(0, C) if h == 0 else slice(C, 2 * C)
        return np.concatenate([w_im_half(wa1[:, sl]), w_im_half(wa2[:, sl]),
                               w_im_half(wa3[:, sl])], axis=-1)

    weights = {
        "w1i": w_im_half(w1),
        "w2i": w_im_half(w2),
        "wqk": np.stack([wqk_half(0), wqk_half(1)], 1),
    }
    in_maps = []
    for b in range(B):
        m = {"x1f": x1f[b], "x2f": x2f[b], "im1": im1[b], "im2": im2[b]}
        m.update(weights)
        in_maps.append(m)
    return in_maps


def kernel(x1, x2, w1, w2, wa1, wa2, wa3):
    from concourse.bass_utils import run_bass_kernel_spmd

    x1 = np.asarray(x1, np.float32)
    x2 = np.asarray(x2, np.float32)
    in_maps = _host_prep(
        x1, x2,
        np.asarray(w1, np.float32), np.asarray(w2, np.float32),
        np.asarray(wa1, np.float32), np.asarray(wa2, np.float32),
        np.asarray(wa3, np.float32),
    )
    nc = _get_nc()
    res = run_bass_kernel_spmd(nc, in_maps, core_ids=list(range(8)))
    out0 = np.stack(
        [np.asarray(res.results[b]["out"], np.float32) for b in range(B)], 0
    ).reshape(B, 2 * C, H, W)
    out1 = np.concatenate([x1, x2], axis=1)
    return out0, out1
